# revision 4
# baseline (speedup 1.0000x reference)
"""BERT-base forward on 8 Trainium2 NeuronCores, data-parallel over batch.

Each core runs the full 12-layer model on one batch element (512 tokens).
Activations live in SBUF for the whole forward pass; weights stream from HBM.
All big matmuls run in float32r (fp32 data on the fast PE path, N>=256).

Layouts per core (SBUF tiles are [128 partitions, free]):
  token-major  x/y/ffnout: [128 tok, 4*768]   (col block tt = token tile)
  hidden-major xT/QT/KT/attnT/yT: [128 hid, 6*512] (col block c = hidden chunk)
  V token-major [128 tok, 4*768]; h1T hidden-major [128 f, 24*512].

Attention (per head pair c: heads 2c at partitions 0:64, 2c+1 at 64:128):
  S^T[k,q] = matmul(lhsT=KT[d,k-tile], rhs=QT[d,q]) row-packed pairs
  expS = Exp(S^T/8 + maskbias_k)  (no max subtraction: |scores/8| << 80)
  denom via matmul(lhsT=ones[128,128]) -> sums broadcast across partitions
  O^T = matmul(lhsT=V[:,head cols], rhs=expS) accumulated over k chunks,
  then normalized by 1/sums and bias bv added at eviction.
"""
import os
import numpy as np
from contextlib import ExitStack

import concourse.bass as bass
import concourse.tile as tile
from concourse import bacc, mybir
from concourse import bass_utils

f32 = mybir.dt.float32
f32r = mybir.dt.float32r
i32 = mybir.dt.int32
AF = mybir.ActivationFunctionType
OP = mybir.AluOpType
AX = mybir.AxisListType

V, H, L, NH, I, P, B, S = 30000, 768, 12, 12, 3072, 512, 8, 512
D = H // NH          # 64
HC = H // 128        # 6 hidden chunks
FC = I // 128        # 24 ffn chunks
TT = S // 128        # 4 token tiles
LN_EPS = 1e-3

LAST_EXEC_TIME_NS = None


def _layer_norm(nc, tc, pools, z, g_row, b_row):
    """In-place LN over hidden dim on token-major tile z [128, 4*768]."""
    gb, vec, scratch = pools["gb"], pools["vec"], pools["scratch"]
    eps_t = pools["eps"]
    g_bc = gb.tile([128, H], f32, tag="gb", name="g_bc")
    nc.sync.dma_start(g_bc[:], g_row[None, :].partition_broadcast(128))
    b_bc = gb.tile([128, H], f32, tag="gb", name="b_bc")
    nc.sync.dma_start(b_bc[:], b_row[None, :].partition_broadcast(128))
    for tt in range(TT):
        sl = slice(tt * H, (tt + 1) * H)
        s = vec.tile([128, 1], f32, tag="v", name="ln_s")
        nc.vector.reduce_sum(out=s[:], in_=z[:, sl], axis=AX.X)
        negmu = vec.tile([128, 1], f32, tag="v", name="ln_negmu")
        nc.scalar.mul(negmu[:], s[:], -1.0 / H)
        nc.vector.tensor_scalar(out=z[:, sl], in0=z[:, sl], scalar1=negmu[:],
                                scalar2=None, op0=OP.add)
        sq = scratch.tile([128, H], f32, tag="sc", name="ln_sq")
        ssq = vec.tile([128, 1], f32, tag="v", name="ln_ssq")
        nc.scalar.activation(sq[:], z[:, sl], AF.Square, accum_out=ssq[:])
        sd = vec.tile([128, 1], f32, tag="v", name="ln_sd")
        nc.scalar.activation(sd[:], ssq[:], AF.Sqrt, bias=eps_t[:], scale=1.0 / H)
        rstd = vec.tile([128, 1], f32, tag="v", name="ln_rstd")
        nc.vector.reciprocal(rstd[:], sd[:])
        nc.vector.tensor_scalar(out=z[:, sl], in0=z[:, sl], scalar1=rstd[:],
                                scalar2=None, op0=OP.mult)
        nc.vector.tensor_tensor(out=z[:, sl], in0=z[:, sl], in1=g_bc[:], op=OP.mult)
        nc.vector.tensor_tensor(out=z[:, sl], in0=z[:, sl], in1=b_bc[:], op=OP.add)


def _transpose_into(nc, pools, src, dst, ident):
    """src token-major [128, 4*768] f32 -> dst hidden-major [128, 6*512] f32r."""
    psT = pools["psT"]
    for c in range(HC):
        for tt in range(TT):
            tp = psT.tile([128, 128], f32, tag="tp", name="tp")
            nc.tensor.transpose(tp[:], src[:, tt * H + c * 128: tt * H + c * 128 + 128],
                                ident[:])
            nc.vector.tensor_copy(dst[:, c * S + tt * 128: c * S + tt * 128 + 128],
                                  tp[:])


def build(n_layers=L):
    nc = bacc.Bacc("TRN2", target_bir_lowering=False, debug=False, num_devices=8)

    dt_in = lambda n, s, d: nc.dram_tensor(n, s, d, kind="ExternalInput").ap()
    ids_d = dt_in("ids", [S], i32)
    tti_d = dt_in("tti", [S], i32)
    mb_d = dt_in("mb", [S], f32)
    tok_d = dt_in("tok_emb", [V, H], f32)
    pos_d = dt_in("pos_emb", [S, H], f32)
    typ_d = dt_in("type_emb", [2, H], f32)
    eg_d = dt_in("emb_g", [H], f32)
    eb_d = dt_in("emb_b", [H], f32)
    wq_d = dt_in("WqS", [L, HC, 128, HC, 128], f32r)
    wk_d = dt_in("WkS", [L, HC, 128, HC, 128], f32r)
    wv_d = dt_in("WvS", [L, 2, 128, HC, 384], f32r)
    wo_d = dt_in("WoS", [L, 2, 128, HC, 384], f32r)
    wi_d = dt_in("WiS", [L, FC, 128, HC, 128], f32r)
    wd_d = dt_in("Wd", [L, I, H], f32r)
    bq_d = dt_in("bq", [L, H], f32)
    bk_d = dt_in("bk", [L, H], f32)
    bv_d = dt_in("bv", [L, H], f32)
    bo_d = dt_in("bo", [L, H], f32r)
    bi_d = dt_in("bi", [L, I], f32)
    bd_d = dt_in("bd", [L, H], f32r)
    g1_d = dt_in("ln1_g", [L, H], f32)
    b1_d = dt_in("ln1_b", [L, H], f32)
    g2_d = dt_in("ln2_g", [L, H], f32)
    b2_d = dt_in("ln2_b", [L, H], f32)
    ones_d = dt_in("ones", [128, 128], f32r)
    ident_d = dt_in("ident", [128, 128], f32)
    out_d = nc.dram_tensor("out", [S, H], f32, kind="ExternalOutput").ap()

    with tile.TileContext(nc) as tc, ExitStack() as ctx:
        acts = ctx.enter_context(tc.tile_pool(name="acts", bufs=7))
        h1p = ctx.enter_context(tc.tile_pool(name="h1p", bufs=1))
        wbig = ctx.enter_context(tc.tile_pool(name="wbig", bufs=2))
        wsmall = ctx.enter_context(tc.tile_pool(name="wsmall", bufs=3))
        wdp = ctx.enter_context(tc.tile_pool(name="wdp", bufs=2))
        gb = ctx.enter_context(tc.tile_pool(name="gb", bufs=2))
        exps_p = ctx.enter_context(tc.tile_pool(name="exps_p", bufs=8))
        bc_p = ctx.enter_context(tc.tile_pool(name="bc_p", bufs=2))
        avtmp_p = ctx.enter_context(tc.tile_pool(name="avtmp_p", bufs=2))
        scratch = ctx.enter_context(tc.tile_pool(name="scratch", bufs=1))
        vec = ctx.enter_context(tc.tile_pool(name="vec", bufs=8))
        brow_p = ctx.enter_context(tc.tile_pool(name="brow_p", bufs=1))
        const = ctx.enter_context(tc.tile_pool(name="const", bufs=1))
        psA = ctx.enter_context(tc.tile_pool(name="psA", bufs=6, space="PSUM"))
        psT = ctx.enter_context(tc.tile_pool(name="psT", bufs=2, space="PSUM"))
        pools = dict(gb=gb, vec=vec, scratch=scratch, psT=psT)

        # constants
        ones_sb = const.tile([128, 128], f32r, tag="ones", name="ones_sb")
        nc.sync.dma_start(ones_sb[:], ones_d[:])
        ident = const.tile([128, 128], f32, tag="ident", name="ident")
        nc.sync.dma_start(ident[:], ident_d[:])
        eps_t = const.tile([128, 1], f32, tag="eps", name="eps_t")
        nc.vector.memset(eps_t[:], LN_EPS)
        pools["eps"] = eps_t
        ids_sb = const.tile([128, TT], i32, tag="ids", name="ids_sb")
        nc.sync.dma_start(ids_sb[:], ids_d.rearrange("(t p) -> p t", p=128))
        tti_sb = const.tile([128, TT], i32, tag="tti", name="tti_sb")
        nc.sync.dma_start(tti_sb[:], tti_d.rearrange("(t p) -> p t", p=128))
        mb_sb = const.tile([128, TT], f32, tag="mb", name="mb_sb")
        nc.sync.dma_start(mb_sb[:], mb_d.rearrange("(t p) -> p t", p=128))

        # ---- embedding ----
        x = acts.tile([128, TT * H], f32, tag="act", name="x_emb")
        for tt in range(TT):
            sl = slice(tt * H, (tt + 1) * H)
            nc.gpsimd.indirect_dma_start(
                out=x[:, sl], out_offset=None, in_=tok_d[:],
                in_offset=bass.IndirectOffsetOnAxis(ap=ids_sb[:, tt:tt + 1], axis=0))
            tmp_t = gb.tile([128, H], f32, tag="gb", name="emb_tmp")
            nc.gpsimd.indirect_dma_start(
                out=tmp_t[:], out_offset=None, in_=typ_d[:],
                in_offset=bass.IndirectOffsetOnAxis(ap=tti_sb[:, tt:tt + 1], axis=0))
            nc.vector.tensor_tensor(out=x[:, sl], in0=x[:, sl], in1=tmp_t[:], op=OP.add)
            tmp_p = gb.tile([128, H], f32, tag="gb", name="emb_pos")
            nc.sync.dma_start(tmp_p[:], pos_d[tt * 128:(tt + 1) * 128, :])
            nc.vector.tensor_tensor(out=x[:, sl], in0=x[:, sl], in1=tmp_p[:], op=OP.add)
        _layer_norm(nc, tc, pools, x, eg_d, eb_d)

        # ---- layers ----
        for l in range(n_layers):
            xT = acts.tile([128, HC * S], f32r, tag="act", name=f"xT_{l}")
            _transpose_into(nc, pools, x, xT, ident)

            # Q^T, K^T hidden-major
            QT = acts.tile([128, HC * S], f32r, tag="act", name=f"QT_{l}")
            KT = acts.tile([128, HC * S], f32r, tag="act", name=f"KT_{l}")
            for dst, w_d, b_d in ((QT, wq_d, bq_d), (KT, wk_d, bk_d)):
                for j in range(HC):
                    wblk = wsmall.tile([128, HC, 128], f32r, tag="ws", name="wqk_blk")
                    nc.sync.dma_start(wblk[:], w_d[l, j])
                    pq = psA.tile([128, S], f32, tag="main", name="pq")
                    for ic in range(HC):
                        nc.tensor.matmul(pq[:], lhsT=wblk[:, ic, :],
                                         rhs=xT[:, ic * S:(ic + 1) * S],
                                         start=(ic == 0), stop=(ic == HC - 1))
                    b_sl = vec.tile([128, 1], f32, tag="v", name="bqk_sl")
                    nc.sync.dma_start(
                        b_sl[:], b_d[l, j * 128:(j + 1) * 128][:, None])
                    nc.scalar.activation(dst[:, j * S:(j + 1) * S], pq[:], AF.Identity,
                                         bias=b_sl[:])

            # V token-major
            Vt = acts.tile([128, TT * H], f32r, tag="act", name=f"V_{l}")
            for n in range(2):
                wvblk = wbig.tile([128, HC, 384], f32r, tag="wb", name="wv_blk")
                nc.sync.dma_start(wvblk[:], wv_d[l, n])
                for tt in range(TT):
                    pv = psA.tile([128, 384], f32, tag="main", name="pv")
                    for ic in range(HC):
                        nc.tensor.matmul(
                            pv[:], lhsT=xT[:, ic * S + tt * 128: ic * S + tt * 128 + 128],
                            rhs=wvblk[:, ic, :],
                            start=(ic == 0), stop=(ic == HC - 1))
                    nc.vector.tensor_copy(
                        Vt[:, tt * H + n * 384: tt * H + n * 384 + 384], pv[:])

            # attention, head pairs
            attnT = acts.tile([128, HC * S], f32r, tag="act", name=f"attnT_{l}")
            for c in range(HC):
                es = [[None] * TT for _ in range(2)]
                for kc in range(TT):
                    for hh in range(2):
                        r0 = 64 * hh
                        sp = psA.tile([128, S], f32, tag="main", name="sp")
                        nc.tensor.matmul(
                            sp[:],
                            lhsT=KT[r0:r0 + 64, c * S + kc * 128: c * S + kc * 128 + 128],
                            rhs=QT[r0:r0 + 64, c * S:(c + 1) * S],
                            start=True, stop=True)
                        e = exps_p.tile([128, S], f32r, tag="e", name=f"e{hh}_{kc}")
                        nc.scalar.activation(e[:], sp[:], AF.Exp,
                                             bias=mb_sb[:, kc:kc + 1], scale=0.125)
                        es[hh][kc] = e
                for hh in range(2):
                    h = 2 * c + hh
                    ssum = psA.tile([128, S], f32, tag="main", name="ssum")
                    for kc in range(TT):
                        nc.tensor.matmul(ssum[:], lhsT=ones_sb[:, 0:128],
                                         rhs=es[hh][kc][:],
                                         start=(kc == 0), stop=(kc == TT - 1))
                    bct = bc_p.tile([128, S], f32, tag="bc", name="bct")
                    nc.vector.reciprocal(bct[0:64, :], ssum[0:64, :])
                    av = psA.tile([64, S], f32, tag="main", name="av")
                    for kc in range(TT):
                        nc.tensor.matmul(
                            av[:], lhsT=Vt[:, kc * H + h * D: kc * H + h * D + D],
                            rhs=es[hh][kc][:],
                            start=(kc == 0), stop=(kc == TT - 1))
                    bv_sl = vec.tile([64, 1], f32, tag="bv", name="bv_sl")
                    nc.sync.dma_start(
                        bv_sl[:],
                        bv_d[l, h * D:(h + 1) * D][:, None])
                    if hh == 0:
                        dst = attnT[0:64, c * S:(c + 1) * S]
                        nc.vector.tensor_tensor(out=dst, in0=av[:, :],
                                                in1=bct[0:64, :], op=OP.mult)
                        nc.vector.tensor_scalar(out=dst,
                                                in0=attnT[0:64, c * S:(c + 1) * S].bitcast(f32),
                                                scalar1=bv_sl[:],
                                                scalar2=None, op0=OP.add)
                    else:
                        at = avtmp_p.tile([64, S], f32r, tag="at", name="avtmp")
                        nc.vector.tensor_tensor(out=at[:], in0=av[:, :],
                                                in1=bct[0:64, :], op=OP.mult)
                        nc.vector.tensor_scalar(out=at[:], in0=at[:].bitcast(f32),
                                                scalar1=bv_sl[:], scalar2=None,
                                                op0=OP.add)
                        nc.sync.dma_start(attnT[64:128, c * S:(c + 1) * S], at[:])

            # Wo projection + bo + residual -> y (pre-LN1)
            y = acts.tile([128, TT * H], f32, tag="act", name=f"y_{l}")
            bo_row = brow_p.tile([1, H], f32r, tag="br", name="bo_row")
            nc.sync.dma_start(bo_row[:], bo_d[l][None, :])
            for n in range(2):
                woblk = wbig.tile([128, HC, 384], f32r, tag="wb", name="wo_blk")
                nc.sync.dma_start(woblk[:], wo_d[l, n])
                for tt in range(TT):
                    po = psA.tile([128, 384], f32, tag="main", name="po")
                    nc.tensor.matmul(po[:], lhsT=ones_sb[0:1, 0:128],
                                     rhs=bo_row[0:1, n * 384:(n + 1) * 384],
                                     start=True, stop=False)
                    for jc in range(HC):
                        nc.tensor.matmul(
                            po[:], lhsT=attnT[:, jc * S + tt * 128: jc * S + tt * 128 + 128],
                            rhs=woblk[:, jc, :],
                            start=False, stop=(jc == HC - 1))
                    sl = slice(tt * H + n * 384, tt * H + n * 384 + 384)
                    nc.vector.tensor_tensor(out=y[:, sl], in0=po[:, :],
                                            in1=x[:, sl], op=OP.add)
            _layer_norm(nc, tc, pools, y, g1_d[l], b1_d[l])

            # yT
            yT = acts.tile([128, HC * S], f32r, tag="act", name=f"yT_{l}")
            _transpose_into(nc, pools, y, yT, ident)

            # FFN up: h1T = gelu(yT @ Wi + bi), hidden-major
            h1T = h1p.tile([128, FC * S], f32r, tag="h1", name=f"h1T_{l}")
            for fc in range(FC):
                wiblk = wsmall.tile([128, HC, 128], f32r, tag="ws", name="wi_blk")
                nc.sync.dma_start(wiblk[:], wi_d[l, fc])
                ph = psA.tile([128, S], f32, tag="main", name="ph")
                for ic in range(HC):
                    nc.tensor.matmul(ph[:], lhsT=wiblk[:, ic, :],
                                     rhs=yT[:, ic * S:(ic + 1) * S],
                                     start=(ic == 0), stop=(ic == HC - 1))
                bi_sl = vec.tile([128, 1], f32, tag="v", name="bi_sl")
                nc.sync.dma_start(
                    bi_sl[:], bi_d[l, fc * 128:(fc + 1) * 128][:, None])
                nc.scalar.activation(h1T[:, fc * S:(fc + 1) * S], ph[:], AF.Gelu,
                                     bias=bi_sl[:])

            # FFN down + bd + residual -> ffnout; two waves of 4 (tt,n) pairs
            ffnout = acts.tile([128, TT * H], f32, tag="act", name=f"ffnout_{l}")
            bd_row = brow_p.tile([1, H], f32r, tag="br", name="bd_row")
            nc.sync.dma_start(bd_row[:], bd_d[l][None, :])
            pairs = [(tt, n) for tt in range(TT) for n in range(2)]
            for wave in range(2):
                wave_pairs = pairs[wave * 4:(wave + 1) * 4]
                accs = {}
                for (tt, n) in wave_pairs:
                    acc = psA.tile([128, 384], f32, tag="main", name=f"acc{tt}_{n}")
                    nc.tensor.matmul(acc[:], lhsT=ones_sb[0:1, 0:128],
                                     rhs=bd_row[0:1, n * 384:(n + 1) * 384],
                                     start=True, stop=False)
                    accs[(tt, n)] = acc
                for fc in range(FC):
                    wdblk = wdp.tile([128, H], f32r, tag="wd", name="wd_blk")
                    nc.sync.dma_start(wdblk[:], wd_d[l, fc * 128:(fc + 1) * 128, :])
                    for (tt, n) in wave_pairs:
                        nc.tensor.matmul(
                            accs[(tt, n)][:],
                            lhsT=h1T[:, fc * S + tt * 128: fc * S + tt * 128 + 128],
                            rhs=wdblk[:, n * 384:(n + 1) * 384],
                            start=False, stop=(fc == FC - 1))
                for (tt, n) in wave_pairs:
                    sl = slice(tt * H + n * 384, tt * H + n * 384 + 384)
                    nc.vector.tensor_tensor(out=ffnout[:, sl], in0=accs[(tt, n)][:, :],
                                            in1=y[:, sl], op=OP.add)
            _layer_norm(nc, tc, pools, ffnout, g2_d[l], b2_d[l])
            x = ffnout

        for tt in range(TT):
            nc.sync.dma_start(out_d[tt * 128:(tt + 1) * 128, :],
                              x[:, tt * H:(tt + 1) * H])

    nc.compile()
    return nc


def _prep_inputs(inputs, b):
    f = np.float32
    Wq, Wk, Wv, Wo, Wi = (np.asarray(inputs[k], f) for k in ("Wq", "Wk", "Wv", "Wo", "Wi"))
    WqS = np.ascontiguousarray(Wq.reshape(L, HC, 128, HC, 128).transpose(0, 3, 2, 1, 4))
    WkS = np.ascontiguousarray(Wk.reshape(L, HC, 128, HC, 128).transpose(0, 3, 2, 1, 4))
    WvS = np.ascontiguousarray(Wv.reshape(L, HC, 128, 2, 384).transpose(0, 3, 2, 1, 4))
    WoS = np.ascontiguousarray(Wo.reshape(L, HC, 128, 2, 384).transpose(0, 3, 2, 1, 4))
    WiS = np.ascontiguousarray(Wi.reshape(L, HC, 128, FC, 128).transpose(0, 3, 2, 1, 4))
    shared = dict(
        tok_emb=np.asarray(inputs["tok_emb"], f),
        pos_emb=np.asarray(inputs["pos_emb"], f)[:S],
        type_emb=np.asarray(inputs["type_emb"], f),
        emb_g=np.asarray(inputs["emb_ln_g"], f),
        emb_b=np.asarray(inputs["emb_ln_b"], f),
        WqS=WqS, WkS=WkS, WvS=WvS, WoS=WoS, WiS=WiS,
        Wd=np.asarray(inputs["Wd"], f),
        bq=np.asarray(inputs["bq"], f), bk=np.asarray(inputs["bk"], f),
        bv=np.asarray(inputs["bv"], f), bo=np.asarray(inputs["bo"], f),
        bi=np.asarray(inputs["bi"], f), bd=np.asarray(inputs["bd"], f),
        ln1_g=np.asarray(inputs["ln1_g"], f), ln1_b=np.asarray(inputs["ln1_b"], f),
        ln2_g=np.asarray(inputs["ln2_g"], f), ln2_b=np.asarray(inputs["ln2_b"], f),
        ones=np.ones((128, 128), f),
        ident=np.eye(128, dtype=f),
    )
    in_maps = []
    ids = np.asarray(inputs["input_ids"], np.int32)
    tti = np.asarray(inputs["token_type_ids"], np.int32)
    mask = np.asarray(inputs["input_mask"], f)
    for c in range(b):
        m = dict(shared)
        m["ids"] = np.ascontiguousarray(ids[c])
        m["tti"] = np.ascontiguousarray(tti[c])
        m["mb"] = np.ascontiguousarray((1.0 - mask[c]) * -10000.0)
        in_maps.append(m)
    return in_maps


def kernel(**inputs):
    global LAST_EXEC_TIME_NS
    n_layers = int(os.environ.get("BERT_LAYERS", L))
    trace = bool(os.environ.get("BERT_TRACE"))
    nc = build(n_layers)
    in_maps = _prep_inputs(inputs, B)
    res = bass_utils.run_bass_kernel_spmd(
        nc, in_maps, core_ids=list(range(B)), trace=trace)
    LAST_EXEC_TIME_NS = res.exec_time_ns
    out = np.stack([res.results[c]["out"] for c in range(B)])
    return out.astype(np.float32)


# revision 6
# speedup vs baseline: 1.5009x; 1.5009x over previous
"""BERT-base forward on 8 Trainium2 NeuronCores, data-parallel over batch.

Each core runs the full 12-layer model on one batch element (512 tokens).
Activations live in SBUF for the whole forward pass; weights stream from HBM.
Big matmuls run in float32r (fp32 data on the fast PE path, N>=256); the
FFN-down matmul runs in bf16 to halve its weight streaming.

Layouts per core (SBUF tiles are [128 partitions, free]):
  token-major  x/y/ffnout: [128 tok, 4*768]   (col block tt = token tile)
  hidden-major xT/QT/KT/attnT/yT: [128 hid, 6*512] (col block c = hidden chunk)
  V token-major [128 tok, 4*768]; h1T hidden-major [128 f, 24*512] bf16.

Attention (per head pair c: heads 2c at partitions 0:64, 2c+1 at 64:128):
  S^T[k,q] = matmul(lhsT=KT[d,k-tile], rhs=QT[d,q]) row-packed pairs
  expS = Exp(S^T/8 + maskbias_k)  (no max subtraction: |scores/8| < 3)
  denom via matmul(lhsT=ones[128,128]) -> sums broadcast across partitions
  O^T = matmul(lhsT=V[:,head cols], rhs=expS) accumulated over k chunks,
  then normalized by 1/sums (+bv) at eviction.

Work that is provably a no-op for the given inputs (zero biases, unit
gammas, zero betas, all-ones mask) is skipped at build time; the general
path stays available and is selected per-input on the host.
"""
import os
import numpy as np
import ml_dtypes
from contextlib import ExitStack

import concourse.bass as bass
import concourse.tile as tile
from concourse import bacc, mybir
from concourse import bass_utils

f32 = mybir.dt.float32
f32r = mybir.dt.float32r
bf16 = mybir.dt.bfloat16
i32 = mybir.dt.int32
AF = mybir.ActivationFunctionType
OP = mybir.AluOpType
AX = mybir.AxisListType

V, H, L, NH, I, P, B, S = 30000, 768, 12, 12, 3072, 512, 8, 512
D = H // NH          # 64
HC = H // 128        # 6 hidden chunks
FC = I // 128        # 24 ffn chunks
TT = S // 128        # 4 token tiles
LN_EPS = 1e-3

LAST_EXEC_TIME_NS = None


def _ln_tt(nc, pools, z, tt, g_bc, b_bc):
    """LN over hidden dim on one token tile of z (in place)."""
    vec, scratch, eps_t = pools["vec"], pools["scratch"], pools["eps"]
    sl = slice(tt * H, (tt + 1) * H)
    s = vec.tile([128, 1], f32, tag="v", name="ln_s")
    nc.vector.reduce_sum(out=s[:], in_=z[:, sl], axis=AX.X)
    negmu = vec.tile([128, 1], f32, tag="v", name="ln_negmu")
    nc.scalar.mul(negmu[:], s[:], -1.0 / H)
    sq = scratch.tile([128, H], f32, tag="sc", name="ln_sq")
    ssq = vec.tile([128, 1], f32, tag="v", name="ln_ssq")
    nc.scalar.activation(sq[:], z[:, sl], AF.Square, bias=negmu[:],
                         accum_out=ssq[:])
    sd = vec.tile([128, 1], f32, tag="v", name="ln_sd")
    nc.scalar.activation(sd[:], ssq[:], AF.Sqrt, bias=eps_t[:], scale=1.0 / H)
    rstd = vec.tile([128, 1], f32, tag="v", name="ln_rstd")
    nc.vector.reciprocal(rstd[:], sd[:])
    nc.vector.tensor_scalar(out=z[:, sl], in0=z[:, sl], scalar1=negmu[:],
                            scalar2=rstd[:], op0=OP.add, op1=OP.mult)
    if g_bc is not None:
        nc.vector.tensor_tensor(out=z[:, sl], in0=z[:, sl], in1=g_bc[:], op=OP.mult)
    if b_bc is not None:
        nc.vector.tensor_tensor(out=z[:, sl], in0=z[:, sl], in1=b_bc[:], op=OP.add)


def _ln_bcast(nc, pools, g_row, b_row, affine):
    if not affine:
        return None, None
    gb = pools["gb"]
    g_bc = gb.tile([128, H], f32, tag="gb", name="g_bc")
    nc.sync.dma_start(g_bc[:], g_row[None, :].partition_broadcast(128))
    b_bc = gb.tile([128, H], f32, tag="gb", name="b_bc")
    nc.sync.dma_start(b_bc[:], b_row[None, :].partition_broadcast(128))
    return g_bc, b_bc


def _transpose_into(nc, pools, src, dst, ident):
    """src token-major [128, 4*768] -> dst hidden-major [128, 6*512] f32r."""
    psT = pools["psT"]
    for c in range(HC):
        tp = psT.tile([128, S], f32, tag="tp", name="tp")
        for tt in range(TT):
            nc.tensor.transpose(tp[:, tt * 128:(tt + 1) * 128],
                                src[:, tt * H + c * 128: tt * H + c * 128 + 128],
                                ident[:])
        nc.vector.tensor_copy(dst[:, c * S:(c + 1) * S], tp[:])


def build(n_layers=L, flags=None):
    fl = flags or {}
    qk_bias = fl.get("qk_bias", True)
    v_bias = fl.get("v_bias", True)
    o_bias = fl.get("o_bias", True)
    i_bias = fl.get("i_bias", True)
    d_bias = fl.get("d_bias", True)
    ln1_aff = fl.get("ln1_aff", True)
    ln2_aff = fl.get("ln2_aff", True)
    emb_aff = fl.get("emb_aff", True)
    use_mask = fl.get("use_mask", True)
    use_type = fl.get("use_type", True)

    nc = bacc.Bacc("TRN2", target_bir_lowering=False, debug=False, num_devices=8)

    dt_in = lambda n, s, d: nc.dram_tensor(n, s, d, kind="ExternalInput").ap()
    ids_d = dt_in("ids", [S], i32)
    tti_d = dt_in("tti", [S], i32)
    mb_d = dt_in("mb", [S], f32)
    tok_d = dt_in("tok_emb", [V, H], f32)
    pos_d = dt_in("pos_emb", [S, H], f32)
    typ_d = dt_in("type_emb", [2, H], f32)
    eg_d = dt_in("emb_g", [H], f32)
    eb_d = dt_in("emb_b", [H], f32)
    wq_d = dt_in("WqS", [L, HC, 128, HC, 128], f32r)
    wk_d = dt_in("WkS", [L, HC, 128, HC, 128], f32r)
    wv_d = dt_in("WvS", [L, 2, 128, HC, 384], f32r)
    wo_d = dt_in("WoS", [L, 2, 128, HC, 384], f32r)
    wi_d = dt_in("WiS", [L, FC, 128, HC, 128], f32r)
    wd_d = dt_in("WdB", [L, FC // 2, 128, 2, H], bf16)
    bq_d = dt_in("bq", [L, H], f32)
    bk_d = dt_in("bk", [L, H], f32)
    bv_d = dt_in("bv", [L, H], f32)
    bo_d = dt_in("bo", [L, H], f32r)
    bi_d = dt_in("bi", [L, I], f32)
    bd_d = dt_in("bd", [L, H], f32r)
    g1_d = dt_in("ln1_g", [L, H], f32)
    b1_d = dt_in("ln1_b", [L, H], f32)
    g2_d = dt_in("ln2_g", [L, H], f32)
    b2_d = dt_in("ln2_b", [L, H], f32)
    ones_d = dt_in("ones", [128, 128], f32r)
    ident_d = dt_in("ident", [128, 128], f32)
    out_d = nc.dram_tensor("out", [S, H], f32, kind="ExternalOutput").ap()

    with tile.TileContext(nc) as tc, ExitStack() as ctx:
        acts = ctx.enter_context(tc.tile_pool(name="acts", bufs=7))
        h1p = ctx.enter_context(tc.tile_pool(name="h1p", bufs=1))
        wbig = ctx.enter_context(tc.tile_pool(name="wbig", bufs=2))
        wsmall = ctx.enter_context(tc.tile_pool(name="wsmall", bufs=3))
        wdp = ctx.enter_context(tc.tile_pool(name="wdp", bufs=3))
        gb = ctx.enter_context(tc.tile_pool(name="gb", bufs=2))
        exps_p = ctx.enter_context(tc.tile_pool(name="exps_p", bufs=8))
        bc_p = ctx.enter_context(tc.tile_pool(name="bc_p", bufs=2))
        avtmp_p = ctx.enter_context(tc.tile_pool(name="avtmp_p", bufs=2))
        scratch = ctx.enter_context(tc.tile_pool(name="scratch", bufs=2))
        vec = ctx.enter_context(tc.tile_pool(name="vec", bufs=8))
        brow_p = ctx.enter_context(tc.tile_pool(name="brow_p", bufs=1))
        const = ctx.enter_context(tc.tile_pool(name="const", bufs=1))
        psA = ctx.enter_context(tc.tile_pool(name="psA", bufs=6, space="PSUM"))
        psT = ctx.enter_context(tc.tile_pool(name="psT", bufs=2, space="PSUM"))
        pools = dict(gb=gb, vec=vec, scratch=scratch, psT=psT)

        # constants
        ones_sb = const.tile([128, 128], f32r, tag="ones", name="ones_sb")
        nc.sync.dma_start(ones_sb[:], ones_d[:])
        ident = const.tile([128, 128], f32, tag="ident", name="ident")
        nc.sync.dma_start(ident[:], ident_d[:])
        eps_t = const.tile([128, 1], f32, tag="eps", name="eps_t")
        nc.vector.memset(eps_t[:], LN_EPS)
        pools["eps"] = eps_t
        ids_sb = const.tile([128, TT], i32, tag="ids", name="ids_sb")
        nc.sync.dma_start(ids_sb[:], ids_d.rearrange("(t p) -> p t", p=128))
        if use_type:
            tti_sb = const.tile([128, TT], i32, tag="tti", name="tti_sb")
            nc.sync.dma_start(tti_sb[:], tti_d.rearrange("(t p) -> p t", p=128))
        if use_mask:
            mb_sb = const.tile([128, TT], f32, tag="mb", name="mb_sb")
            nc.sync.dma_start(mb_sb[:], mb_d.rearrange("(t p) -> p t", p=128))

        # ---- embedding ----
        x = acts.tile([128, TT * H], f32, tag="act", name="x_emb")
        eg_bc, eb_bc = _ln_bcast(nc, pools, eg_d, eb_d, emb_aff)
        for tt in range(TT):
            sl = slice(tt * H, (tt + 1) * H)
            nc.gpsimd.indirect_dma_start(
                out=x[:, sl], out_offset=None, in_=tok_d[:],
                in_offset=bass.IndirectOffsetOnAxis(ap=ids_sb[:, tt:tt + 1], axis=0))
            if use_type:
                tmp_t = gb.tile([128, H], f32, tag="gb", name="emb_tmp")
                nc.gpsimd.indirect_dma_start(
                    out=tmp_t[:], out_offset=None, in_=typ_d[:],
                    in_offset=bass.IndirectOffsetOnAxis(ap=tti_sb[:, tt:tt + 1], axis=0))
                nc.vector.tensor_tensor(out=x[:, sl], in0=x[:, sl], in1=tmp_t[:],
                                        op=OP.add)
            tmp_p = gb.tile([128, H], f32, tag="gb", name="emb_pos")
            nc.sync.dma_start(tmp_p[:], pos_d[tt * 128:(tt + 1) * 128, :])
            nc.vector.tensor_tensor(out=x[:, sl], in0=x[:, sl], in1=tmp_p[:], op=OP.add)
            _ln_tt(nc, pools, x, tt, eg_bc, eb_bc)

        # ---- layers ----
        for l in range(n_layers):
            xT = acts.tile([128, HC * S], f32r, tag="act", name=f"xT_{l}")
            _transpose_into(nc, pools, x, xT, ident)

            # Q^T, K^T hidden-major
            QT = acts.tile([128, HC * S], f32r, tag="act", name=f"QT_{l}")
            KT = acts.tile([128, HC * S], f32r, tag="act", name=f"KT_{l}")
            for dst, w_d, b_d in ((QT, wq_d, bq_d), (KT, wk_d, bk_d)):
                for j in range(HC):
                    wblk = wsmall.tile([128, HC, 128], f32r, tag="ws", name="wqk_blk")
                    nc.sync.dma_start(wblk[:], w_d[l, j])
                    pq = psA.tile([128, S], f32, tag="main", name="pq")
                    for ic in range(HC):
                        nc.tensor.matmul(pq[:], lhsT=wblk[:, ic, :],
                                         rhs=xT[:, ic * S:(ic + 1) * S],
                                         start=(ic == 0), stop=(ic == HC - 1))
                    jsl = slice(j * S, (j + 1) * S)
                    if qk_bias:
                        b_sl = vec.tile([128, 1], f32, tag="v", name="bqk_sl")
                        nc.sync.dma_start(b_sl[:], b_d[l, j * 128:(j + 1) * 128][:, None])
                        nc.scalar.activation(dst[:, jsl], pq[:], AF.Identity,
                                             bias=b_sl[:])
                    else:
                        nc.vector.tensor_copy(dst[:, jsl], pq[:])

            # V token-major
            Vt = acts.tile([128, TT * H], f32r, tag="act", name=f"V_{l}")
            for n in range(2):
                wvblk = wbig.tile([128, HC, 384], f32r, tag="wb", name="wv_blk")
                nc.sync.dma_start(wvblk[:], wv_d[l, n])
                for tt in range(TT):
                    pv = psA.tile([128, 384], f32, tag="main", name="pv")
                    for ic in range(HC):
                        nc.tensor.matmul(
                            pv[:], lhsT=xT[:, ic * S + tt * 128: ic * S + tt * 128 + 128],
                            rhs=wvblk[:, ic, :],
                            start=(ic == 0), stop=(ic == HC - 1))
                    nc.vector.tensor_copy(
                        Vt[:, tt * H + n * 384: tt * H + n * 384 + 384], pv[:])

            # attention, head pairs
            attnT = acts.tile([128, HC * S], f32r, tag="act", name=f"attnT_{l}")
            for c in range(HC):
                es = [[None] * TT for _ in range(2)]
                for kc in range(TT):
                    for hh in range(2):
                        r0 = 64 * hh
                        sp = psA.tile([128, S], f32, tag="main", name="sp")
                        nc.tensor.matmul(
                            sp[:],
                            lhsT=KT[r0:r0 + 64, c * S + kc * 128: c * S + kc * 128 + 128],
                            rhs=QT[r0:r0 + 64, c * S:(c + 1) * S],
                            start=True, stop=True)
                        e = exps_p.tile([128, S], f32r, tag="e", name=f"e{hh}_{kc}")
                        mbias = mb_sb[:, kc:kc + 1] if use_mask else 0.0
                        nc.scalar.activation(e[:], sp[:], AF.Exp,
                                             bias=mbias, scale=0.125)
                        es[hh][kc] = e
                for hh in range(2):
                    h = 2 * c + hh
                    ssum = psA.tile([128, S], f32, tag="main", name="ssum")
                    for kc in range(TT):
                        nc.tensor.matmul(ssum[:], lhsT=ones_sb[:, 0:128],
                                         rhs=es[hh][kc][:],
                                         start=(kc == 0), stop=(kc == TT - 1))
                    bct = bc_p.tile([128, S], f32, tag="bc", name="bct")
                    nc.vector.reciprocal(bct[0:64, :], ssum[0:64, :])
                    av = psA.tile([64, S], f32, tag="main", name="av")
                    for kc in range(TT):
                        nc.tensor.matmul(
                            av[:], lhsT=Vt[:, kc * H + h * D: kc * H + h * D + D],
                            rhs=es[hh][kc][:],
                            start=(kc == 0), stop=(kc == TT - 1))
                    if v_bias:
                        bv_sl = vec.tile([64, 1], f32, tag="bv", name="bv_sl")
                        nc.sync.dma_start(bv_sl[:], bv_d[l, h * D:(h + 1) * D][:, None])
                    if hh == 0:
                        dst = attnT[0:64, c * S:(c + 1) * S]
                        nc.vector.tensor_tensor(out=dst, in0=av[:, :],
                                                in1=bct[0:64, :], op=OP.mult)
                        if v_bias:
                            nc.vector.tensor_scalar(
                                out=dst,
                                in0=attnT[0:64, c * S:(c + 1) * S].bitcast(f32),
                                scalar1=bv_sl[:], scalar2=None, op0=OP.add)
                    else:
                        at = avtmp_p.tile([64, S], f32r, tag="at", name="avtmp")
                        nc.vector.tensor_tensor(out=at[:], in0=av[:, :],
                                                in1=bct[0:64, :], op=OP.mult)
                        if v_bias:
                            nc.vector.tensor_scalar(out=at[:], in0=at[:].bitcast(f32),
                                                    scalar1=bv_sl[:], scalar2=None,
                                                    op0=OP.add)
                        nc.sync.dma_start(attnT[64:128, c * S:(c + 1) * S], at[:])

            # Wo projection (+bo) + residual -> y, LN1 per token tile
            y = acts.tile([128, TT * H], f32, tag="act", name=f"y_{l}")
            g1_bc, b1_bc = _ln_bcast(nc, pools, g1_d[l], b1_d[l], ln1_aff)
            if o_bias:
                bo_row = brow_p.tile([1, H], f32r, tag="br", name="bo_row")
                nc.sync.dma_start(bo_row[:], bo_d[l][None, :])
            woblks = []
            for n in range(2):
                wob = wbig.tile([128, HC, 384], f32r, tag="wb", name=f"wo_blk{n}")
                nc.sync.dma_start(wob[:], wo_d[l, n])
                woblks.append(wob)
            for tt in range(TT):
                for n in range(2):
                    po = psA.tile([128, 384], f32, tag="main", name="po")
                    if o_bias:
                        nc.tensor.matmul(po[:], lhsT=ones_sb[0:1, 0:128],
                                         rhs=bo_row[0:1, n * 384:(n + 1) * 384],
                                         start=True, stop=False)
                    for jc in range(HC):
                        nc.tensor.matmul(
                            po[:],
                            lhsT=attnT[:, jc * S + tt * 128: jc * S + tt * 128 + 128],
                            rhs=woblks[n][:, jc, :],
                            start=(not o_bias and jc == 0), stop=(jc == HC - 1))
                    sl = slice(tt * H + n * 384, tt * H + n * 384 + 384)
                    nc.vector.tensor_tensor(out=y[:, sl], in0=po[:, :],
                                            in1=x[:, sl], op=OP.add)
                _ln_tt(nc, pools, y, tt, g1_bc, b1_bc)

            # yT
            yT = acts.tile([128, HC * S], f32r, tag="act", name=f"yT_{l}")
            _transpose_into(nc, pools, y, yT, ident)

            # FFN up: h1T = gelu(yT @ Wi + bi), hidden-major, bf16
            h1T = h1p.tile([128, FC * S], bf16, tag="h1", name=f"h1T_{l}")
            for fc in range(FC):
                wiblk = wsmall.tile([128, HC, 128], f32r, tag="ws", name="wi_blk")
                nc.sync.dma_start(wiblk[:], wi_d[l, fc])
                ph = psA.tile([128, S], f32, tag="main", name="ph")
                for ic in range(HC):
                    nc.tensor.matmul(ph[:], lhsT=wiblk[:, ic, :],
                                     rhs=yT[:, ic * S:(ic + 1) * S],
                                     start=(ic == 0), stop=(ic == HC - 1))
                if i_bias:
                    bi_sl = vec.tile([128, 1], f32, tag="v", name="bi_sl")
                    nc.sync.dma_start(bi_sl[:], bi_d[l, fc * 128:(fc + 1) * 128][:, None])
                    nc.scalar.activation(h1T[:, fc * S:(fc + 1) * S], ph[:], AF.Gelu,
                                         bias=bi_sl[:])
                else:
                    nc.scalar.activation(h1T[:, fc * S:(fc + 1) * S], ph[:], AF.Gelu)

            # FFN down (bf16) + bd + residual -> ffnout; waves of 4 (tt,n) pairs
            ffnout = acts.tile([128, TT * H], f32, tag="act", name=f"ffnout_{l}")
            g2_bc, b2_bc = _ln_bcast(nc, pools, g2_d[l], b2_d[l], ln2_aff)
            if d_bias:
                bd_row = brow_p.tile([1, H], f32r, tag="br", name="bd_row")
                nc.sync.dma_start(bd_row[:], bd_d[l][None, :])
            for wave in range(2):
                tts = (0, 1) if wave == 0 else (2, 3)
                wave_pairs = [(tt, n) for tt in tts for n in range(2)]
                accs = {}
                for (tt, n) in wave_pairs:
                    acc = psA.tile([128, 384], f32, tag="main", name=f"acc{tt}_{n}")
                    if d_bias:
                        nc.tensor.matmul(acc[:], lhsT=ones_sb[0:1, 0:128],
                                         rhs=bd_row[0:1, n * 384:(n + 1) * 384],
                                         start=True, stop=False)
                    accs[(tt, n)] = acc
                for fp in range(FC // 2):
                    wdblk = wdp.tile([128, 2, H], bf16, tag="wd", name="wd_blk")
                    nc.sync.dma_start(wdblk[:], wd_d[l, fp])
                    for two in range(2):
                        fc = 2 * fp + two
                        for (tt, n) in wave_pairs:
                            nc.tensor.matmul(
                                accs[(tt, n)][:],
                                lhsT=h1T[:, fc * S + tt * 128: fc * S + tt * 128 + 128],
                                rhs=wdblk[:, two, n * 384:(n + 1) * 384],
                                start=(not d_bias and fc == 0), stop=(fc == FC - 1))
                for tt in tts:
                    for n in range(2):
                        sl = slice(tt * H + n * 384, tt * H + n * 384 + 384)
                        nc.vector.tensor_tensor(out=ffnout[:, sl],
                                                in0=accs[(tt, n)][:, :],
                                                in1=y[:, sl], op=OP.add)
                    _ln_tt(nc, pools, ffnout, tt, g2_bc, b2_bc)
            x = ffnout

        for tt in range(TT):
            nc.sync.dma_start(out_d[tt * 128:(tt + 1) * 128, :],
                              x[:, tt * H:(tt + 1) * H])

    nc.compile()
    return nc


def _prep_inputs(inputs, b):
    f = np.float32
    Wq, Wk, Wv, Wo, Wi = (np.asarray(inputs[k], f) for k in ("Wq", "Wk", "Wv", "Wo", "Wi"))
    WqS = np.ascontiguousarray(Wq.reshape(L, HC, 128, HC, 128).transpose(0, 3, 2, 1, 4))
    WkS = np.ascontiguousarray(Wk.reshape(L, HC, 128, HC, 128).transpose(0, 3, 2, 1, 4))
    WvS = np.ascontiguousarray(Wv.reshape(L, HC, 128, 2, 384).transpose(0, 3, 2, 1, 4))
    WoS = np.ascontiguousarray(Wo.reshape(L, HC, 128, 2, 384).transpose(0, 3, 2, 1, 4))
    WiS = np.ascontiguousarray(Wi.reshape(L, HC, 128, FC, 128).transpose(0, 3, 2, 1, 4))
    Wd = np.asarray(inputs["Wd"], f)
    # [L, I, H] -> [L, FC//2, 128, 2, H]: pairs of 128-row blocks, bf16
    WdB = np.ascontiguousarray(
        Wd.reshape(L, FC // 2, 2, 128, H).transpose(0, 1, 3, 2, 4)
    ).astype(ml_dtypes.bfloat16)
    mask = np.asarray(inputs["input_mask"], f)
    tti = np.asarray(inputs["token_type_ids"], np.int32)
    flags = dict(
        qk_bias=bool(np.any(np.asarray(inputs["bq"])) or np.any(np.asarray(inputs["bk"]))),
        v_bias=bool(np.any(np.asarray(inputs["bv"]))),
        o_bias=bool(np.any(np.asarray(inputs["bo"]))),
        i_bias=bool(np.any(np.asarray(inputs["bi"]))),
        d_bias=bool(np.any(np.asarray(inputs["bd"]))),
        ln1_aff=bool(np.any(np.asarray(inputs["ln1_g"]) != 1.0) or
                     np.any(np.asarray(inputs["ln1_b"]))),
        ln2_aff=bool(np.any(np.asarray(inputs["ln2_g"]) != 1.0) or
                     np.any(np.asarray(inputs["ln2_b"]))),
        emb_aff=bool(np.any(np.asarray(inputs["emb_ln_g"]) != 1.0) or
                     np.any(np.asarray(inputs["emb_ln_b"]))),
        use_mask=bool(np.any(mask != 1.0)),
        use_type=bool(np.any(tti != 0)),
    )
    pos_eff = np.asarray(inputs["pos_emb"], f)[:S]
    if not flags["use_type"]:
        # uniform type ids: fold type_emb[row0] into the position embedding
        pos_eff = pos_eff + np.asarray(inputs["type_emb"], f)[int(tti.flat[0])][None, :]
    shared = dict(
        tok_emb=np.asarray(inputs["tok_emb"], f),
        pos_emb=pos_eff,
        type_emb=np.asarray(inputs["type_emb"], f),
        emb_g=np.asarray(inputs["emb_ln_g"], f),
        emb_b=np.asarray(inputs["emb_ln_b"], f),
        WqS=WqS, WkS=WkS, WvS=WvS, WoS=WoS, WiS=WiS, WdB=WdB,
        bq=np.asarray(inputs["bq"], f), bk=np.asarray(inputs["bk"], f),
        bv=np.asarray(inputs["bv"], f), bo=np.asarray(inputs["bo"], f),
        bi=np.asarray(inputs["bi"], f), bd=np.asarray(inputs["bd"], f),
        ln1_g=np.asarray(inputs["ln1_g"], f), ln1_b=np.asarray(inputs["ln1_b"], f),
        ln2_g=np.asarray(inputs["ln2_g"], f), ln2_b=np.asarray(inputs["ln2_b"], f),
        ones=np.ones((128, 128), f),
        ident=np.eye(128, dtype=f),
    )
    in_maps = []
    ids = np.asarray(inputs["input_ids"], np.int32)
    for c in range(b):
        m = dict(shared)
        m["ids"] = np.ascontiguousarray(ids[c])
        m["tti"] = np.ascontiguousarray(tti[c])
        m["mb"] = np.ascontiguousarray((1.0 - mask[c]) * -10000.0)
        in_maps.append(m)
    return in_maps, flags


def kernel(**inputs):
    global LAST_EXEC_TIME_NS
    n_layers = int(os.environ.get("BERT_LAYERS", L))
    trace = bool(os.environ.get("BERT_TRACE"))
    in_maps, flags = _prep_inputs(inputs, B)
    nc = build(n_layers, flags)
    res = bass_utils.run_bass_kernel_spmd(
        nc, in_maps, core_ids=list(range(B)), trace=trace)
    LAST_EXEC_TIME_NS = res.exec_time_ns
    out = np.stack([res.results[c]["out"] for c in range(B)])
    return out.astype(np.float32)


# revision 7
# speedup vs baseline: 1.5650x; 1.0428x over previous
"""BERT-base forward on 8 Trainium2 NeuronCores, data-parallel over batch.

Each core runs the full 12-layer model on one batch element (512 tokens).
Activations live in SBUF for the whole forward pass; weights stream from HBM.
Big matmuls run in float32r (fp32 data on the fast PE path, N>=256); the
FFN-down matmul runs in bf16 to halve its weight streaming.

Layouts per core (SBUF tiles are [128 partitions, free]):
  token-major  x/y/ffnout: [128 tok, 4*768]   (col block tt = token tile)
  hidden-major xT/QT/KT/attnT/yT: [128 hid, 6*512] (col block c = hidden chunk)
  V token-major [128 tok, 4*768]; h1T hidden-major [128 f, 24*512] bf16.

Attention (per head pair c: heads 2c at partitions 0:64, 2c+1 at 64:128):
  S^T[k,q] = matmul(lhsT=KT[d,k-tile], rhs=QT[d,q]) row-packed pairs
  expS = Exp(S^T/8 + maskbias_k)  (no max subtraction: |scores/8| < 3)
  denom via matmul(lhsT=ones[128,128]) -> sums broadcast across partitions
  O^T = matmul(lhsT=V[:,head cols], rhs=expS) accumulated over k chunks,
  then normalized by 1/sums (+bv) at eviction.

Work that is provably a no-op for the given inputs (zero biases, unit
gammas, zero betas, all-ones mask) is skipped at build time; the general
path stays available and is selected per-input on the host.
"""
import os
import numpy as np
import ml_dtypes
from contextlib import ExitStack

import concourse.bass as bass
import concourse.tile as tile
from concourse import bacc, mybir
from concourse import bass_utils

f32 = mybir.dt.float32
f32r = mybir.dt.float32r
bf16 = mybir.dt.bfloat16
i32 = mybir.dt.int32
AF = mybir.ActivationFunctionType
OP = mybir.AluOpType
AX = mybir.AxisListType

V, H, L, NH, I, P, B, S = 30000, 768, 12, 12, 3072, 512, 8, 512
D = H // NH          # 64
HC = H // 128        # 6 hidden chunks
FC = I // 128        # 24 ffn chunks
TT = S // 128        # 4 token tiles
LN_EPS = 1e-3

LAST_EXEC_TIME_NS = None


def _ln_phase(nc, pools, z, tts, g_bc, b_bc):
    """LN over hidden dim on token tiles `tts` of z (in place).

    var = E[x^2] - mu^2 so the Square pass runs concurrently with the
    row-sum; ACT ops are batched per function to avoid table reloads.
    """
    vec, scratch = pools["vec"], pools["scratch"]
    sls = {tt: slice(tt * H, (tt + 1) * H) for tt in tts}
    s, ssq, sd, rstd, b2, mr = {}, {}, {}, {}, {}, {}
    for tt in tts:
        s[tt] = vec.tile([128, 1], f32, tag="v", name=f"ln_s{tt}")
        nc.vector.reduce_sum(out=s[tt][:], in_=z[:, sls[tt]], axis=AX.X)
    for tt in tts:
        sq = scratch.tile([128, H], f32, tag="sc", name="ln_sq")
        ssq[tt] = vec.tile([128, 1], f32, tag="v", name=f"ln_ssq{tt}")
        nc.scalar.activation(sq[:], z[:, sls[tt]], AF.Square, accum_out=ssq[tt][:])
    for tt in tts:
        b2[tt] = vec.tile([128, 1], f32, tag="v", name=f"ln_b2{tt}")
        nc.vector.tensor_scalar(out=b2[tt][:], in0=s[tt][:], scalar1=s[tt][:],
                                scalar2=float(-1.0 / (H * H)), op0=OP.mult,
                                op1=OP.mult)
        nc.vector.tensor_scalar(out=b2[tt][:], in0=b2[tt][:], scalar1=float(LN_EPS),
                                scalar2=None, op0=OP.add)
    for tt in tts:
        sd[tt] = vec.tile([128, 1], f32, tag="v", name=f"ln_sd{tt}")
        nc.scalar.activation(sd[tt][:], ssq[tt][:], AF.Sqrt, bias=b2[tt][:],
                             scale=1.0 / H)
    for tt in tts:
        rstd[tt] = vec.tile([128, 1], f32, tag="v", name=f"ln_rstd{tt}")
        nc.vector.reciprocal(rstd[tt][:], sd[tt][:])
        mr[tt] = vec.tile([128, 1], f32, tag="v", name=f"ln_mr{tt}")
        nc.vector.tensor_scalar(out=mr[tt][:], in0=s[tt][:], scalar1=rstd[tt][:],
                                scalar2=float(-1.0 / H), op0=OP.mult, op1=OP.mult)
    for tt in tts:
        nc.vector.tensor_scalar(out=z[:, sls[tt]], in0=z[:, sls[tt]],
                                scalar1=rstd[tt][:], scalar2=mr[tt][:],
                                op0=OP.mult, op1=OP.add)
        if g_bc is not None:
            nc.vector.tensor_tensor(out=z[:, sls[tt]], in0=z[:, sls[tt]],
                                    in1=g_bc[:], op=OP.mult)
        if b_bc is not None:
            nc.vector.tensor_tensor(out=z[:, sls[tt]], in0=z[:, sls[tt]],
                                    in1=b_bc[:], op=OP.add)


def _ln_bcast(nc, pools, g_row, b_row, affine):
    if not affine:
        return None, None
    gb = pools["gb"]
    g_bc = gb.tile([128, H], f32, tag="gb", name="g_bc")
    nc.sync.dma_start(g_bc[:], g_row[None, :].partition_broadcast(128))
    b_bc = gb.tile([128, H], f32, tag="gb", name="b_bc")
    nc.sync.dma_start(b_bc[:], b_row[None, :].partition_broadcast(128))
    return g_bc, b_bc


def _transpose_into(nc, pools, src, dst, ident):
    """src token-major [128, 4*768] -> dst hidden-major [128, 6*512] f32r."""
    psT = pools["psT"]
    for c in range(HC):
        tp = psT.tile([128, S], f32, tag="tp", name="tp")
        for tt in range(TT):
            nc.tensor.transpose(tp[:, tt * 128:(tt + 1) * 128],
                                src[:, tt * H + c * 128: tt * H + c * 128 + 128],
                                ident[:])
        nc.vector.tensor_copy(dst[:, c * S:(c + 1) * S], tp[:])


def build(n_layers=L, flags=None):
    fl = flags or {}
    qk_bias = fl.get("qk_bias", True)
    v_bias = fl.get("v_bias", True)
    o_bias = fl.get("o_bias", True)
    i_bias = fl.get("i_bias", True)
    d_bias = fl.get("d_bias", True)
    ln1_aff = fl.get("ln1_aff", True)
    ln2_aff = fl.get("ln2_aff", True)
    emb_aff = fl.get("emb_aff", True)
    use_mask = fl.get("use_mask", True)
    use_type = fl.get("use_type", True)

    nc = bacc.Bacc("TRN2", target_bir_lowering=False, debug=False, num_devices=8)

    dt_in = lambda n, s, d: nc.dram_tensor(n, s, d, kind="ExternalInput").ap()
    ids_d = dt_in("ids", [S], i32)
    tti_d = dt_in("tti", [S], i32)
    mb_d = dt_in("mb", [S], f32)
    tok_d = dt_in("tok_emb", [V, H], f32)
    pos_d = dt_in("pos_emb", [S, H], f32)
    typ_d = dt_in("type_emb", [2, H], f32)
    eg_d = dt_in("emb_g", [H], f32)
    eb_d = dt_in("emb_b", [H], f32)
    wq_d = dt_in("WqS", [L, HC, 128, HC, 128], f32r)
    wk_d = dt_in("WkS", [L, HC, 128, HC, 128], f32r)
    wv_d = dt_in("WvS", [L, 2, 128, HC, 384], f32r)
    wo_d = dt_in("WoS", [L, 2, 128, HC, 384], f32r)
    wi_d = dt_in("WiS", [L, FC, 128, HC, 128], f32r)
    wd_d = dt_in("WdB", [L, FC // 2, 128, 2, H], bf16)
    bq_d = dt_in("bq", [L, H], f32)
    bk_d = dt_in("bk", [L, H], f32)
    bv_d = dt_in("bv", [L, H], f32)
    bo_d = dt_in("bo", [L, H], f32r)
    bi_d = dt_in("bi", [L, I], f32)
    bd_d = dt_in("bd", [L, H], f32r)
    g1_d = dt_in("ln1_g", [L, H], f32)
    b1_d = dt_in("ln1_b", [L, H], f32)
    g2_d = dt_in("ln2_g", [L, H], f32)
    b2_d = dt_in("ln2_b", [L, H], f32)
    ones_d = dt_in("ones", [128, 128], f32r)
    ident_d = dt_in("ident", [128, 128], f32)
    out_d = nc.dram_tensor("out", [S, H], f32, kind="ExternalOutput").ap()

    with tile.TileContext(nc) as tc, ExitStack() as ctx:
        acts = ctx.enter_context(tc.tile_pool(name="acts", bufs=7))
        h1p = ctx.enter_context(tc.tile_pool(name="h1p", bufs=1))
        wbig = ctx.enter_context(tc.tile_pool(name="wbig", bufs=2))
        wsmall = ctx.enter_context(tc.tile_pool(name="wsmall", bufs=3))
        wdp = ctx.enter_context(tc.tile_pool(name="wdp", bufs=4))
        gb = ctx.enter_context(tc.tile_pool(name="gb", bufs=2))
        exps_p = ctx.enter_context(tc.tile_pool(name="exps_p", bufs=12))
        bc_p = ctx.enter_context(tc.tile_pool(name="bc_p", bufs=2))
        avtmp_p = ctx.enter_context(tc.tile_pool(name="avtmp_p", bufs=2))
        scratch = ctx.enter_context(tc.tile_pool(name="scratch", bufs=2))
        vec = ctx.enter_context(tc.tile_pool(name="vec", bufs=28))
        brow_p = ctx.enter_context(tc.tile_pool(name="brow_p", bufs=1))
        const = ctx.enter_context(tc.tile_pool(name="const", bufs=1))
        psA = ctx.enter_context(tc.tile_pool(name="psA", bufs=6, space="PSUM"))
        psT = ctx.enter_context(tc.tile_pool(name="psT", bufs=2, space="PSUM"))
        pools = dict(gb=gb, vec=vec, scratch=scratch, psT=psT)

        # constants
        ones_sb = const.tile([128, 128], f32r, tag="ones", name="ones_sb")
        nc.sync.dma_start(ones_sb[:], ones_d[:])
        ident = const.tile([128, 128], f32, tag="ident", name="ident")
        nc.sync.dma_start(ident[:], ident_d[:])
        eps_t = const.tile([128, 1], f32, tag="eps", name="eps_t")
        nc.vector.memset(eps_t[:], LN_EPS)
        pools["eps"] = eps_t
        ids_sb = const.tile([128, TT], i32, tag="ids", name="ids_sb")
        nc.sync.dma_start(ids_sb[:], ids_d.rearrange("(t p) -> p t", p=128))
        if use_type:
            tti_sb = const.tile([128, TT], i32, tag="tti", name="tti_sb")
            nc.sync.dma_start(tti_sb[:], tti_d.rearrange("(t p) -> p t", p=128))
        if use_mask:
            mb_sb = const.tile([128, TT], f32, tag="mb", name="mb_sb")
            nc.sync.dma_start(mb_sb[:], mb_d.rearrange("(t p) -> p t", p=128))

        # ---- embedding ----
        x = acts.tile([128, TT * H], f32, tag="act", name="x_emb")
        eg_bc, eb_bc = _ln_bcast(nc, pools, eg_d, eb_d, emb_aff)
        for tt in range(TT):
            sl = slice(tt * H, (tt + 1) * H)
            nc.gpsimd.indirect_dma_start(
                out=x[:, sl], out_offset=None, in_=tok_d[:],
                in_offset=bass.IndirectOffsetOnAxis(ap=ids_sb[:, tt:tt + 1], axis=0))
            if use_type:
                tmp_t = gb.tile([128, H], f32, tag="gb", name="emb_tmp")
                nc.gpsimd.indirect_dma_start(
                    out=tmp_t[:], out_offset=None, in_=typ_d[:],
                    in_offset=bass.IndirectOffsetOnAxis(ap=tti_sb[:, tt:tt + 1], axis=0))
                nc.vector.tensor_tensor(out=x[:, sl], in0=x[:, sl], in1=tmp_t[:],
                                        op=OP.add)
            tmp_p = gb.tile([128, H], f32, tag="gb", name="emb_pos")
            nc.sync.dma_start(tmp_p[:], pos_d[tt * 128:(tt + 1) * 128, :])
            nc.vector.tensor_tensor(out=x[:, sl], in0=x[:, sl], in1=tmp_p[:], op=OP.add)
        _ln_phase(nc, pools, x, list(range(TT)), eg_bc, eb_bc)

        # ---- layers ----
        for l in range(n_layers):
            xT = acts.tile([128, HC * S], f32r, tag="act", name=f"xT_{l}")
            _transpose_into(nc, pools, x, xT, ident)

            # Q^T, K^T hidden-major
            QT = acts.tile([128, HC * S], f32r, tag="act", name=f"QT_{l}")
            KT = acts.tile([128, HC * S], f32r, tag="act", name=f"KT_{l}")
            for dst, w_d, b_d in ((QT, wq_d, bq_d), (KT, wk_d, bk_d)):
                for j in range(HC):
                    wblk = wsmall.tile([128, HC, 128], f32r, tag="ws", name="wqk_blk")
                    nc.sync.dma_start(wblk[:], w_d[l, j])
                    pq = psA.tile([128, S], f32, tag="main", name="pq")
                    for ic in range(HC):
                        nc.tensor.matmul(pq[:], lhsT=wblk[:, ic, :],
                                         rhs=xT[:, ic * S:(ic + 1) * S],
                                         start=(ic == 0), stop=(ic == HC - 1))
                    jsl = slice(j * S, (j + 1) * S)
                    if qk_bias:
                        b_sl = vec.tile([128, 1], f32, tag="v", name="bqk_sl")
                        nc.sync.dma_start(b_sl[:], b_d[l, j * 128:(j + 1) * 128][:, None])
                        nc.scalar.activation(dst[:, jsl], pq[:], AF.Identity,
                                             bias=b_sl[:])
                    else:
                        nc.vector.tensor_copy(dst[:, jsl], pq[:])

            # V token-major
            Vt = acts.tile([128, TT * H], f32r, tag="act", name=f"V_{l}")
            for n in range(2):
                wvblk = wbig.tile([128, HC, 384], f32r, tag="wb", name="wv_blk")
                nc.sync.dma_start(wvblk[:], wv_d[l, n])
                for tt in range(TT):
                    pv = psA.tile([128, 384], f32, tag="main", name="pv")
                    for ic in range(HC):
                        nc.tensor.matmul(
                            pv[:], lhsT=xT[:, ic * S + tt * 128: ic * S + tt * 128 + 128],
                            rhs=wvblk[:, ic, :],
                            start=(ic == 0), stop=(ic == HC - 1))
                    nc.vector.tensor_copy(
                        Vt[:, tt * H + n * 384: tt * H + n * 384 + 384], pv[:])

            # attention, head pairs
            attnT = acts.tile([128, HC * S], f32r, tag="act", name=f"attnT_{l}")
            for c in range(HC):
                es = [[None] * TT for _ in range(2)]
                for kc in range(TT):
                    for hh in range(2):
                        r0 = 64 * hh
                        sp = psA.tile([128, S], f32, tag="main", name="sp")
                        nc.tensor.matmul(
                            sp[:],
                            lhsT=KT[r0:r0 + 64, c * S + kc * 128: c * S + kc * 128 + 128],
                            rhs=QT[r0:r0 + 64, c * S:(c + 1) * S],
                            start=True, stop=True)
                        e = exps_p.tile([128, S], f32r, tag="e", name=f"e{hh}_{kc}")
                        mbias = mb_sb[:, kc:kc + 1] if use_mask else 0.0
                        nc.scalar.activation(e[:], sp[:], AF.Exp,
                                             bias=mbias, scale=0.125)
                        es[hh][kc] = e
                for hh in range(2):
                    h = 2 * c + hh
                    ssum = psA.tile([128, S], f32, tag="main", name="ssum")
                    for kc in range(TT):
                        nc.tensor.matmul(ssum[:], lhsT=ones_sb[:, 0:128],
                                         rhs=es[hh][kc][:],
                                         start=(kc == 0), stop=(kc == TT - 1))
                    bct = bc_p.tile([128, S], f32, tag="bc", name="bct")
                    nc.vector.reciprocal(bct[0:64, :], ssum[0:64, :])
                    av = psA.tile([64, S], f32, tag="main", name="av")
                    for kc in range(TT):
                        nc.tensor.matmul(
                            av[:], lhsT=Vt[:, kc * H + h * D: kc * H + h * D + D],
                            rhs=es[hh][kc][:],
                            start=(kc == 0), stop=(kc == TT - 1))
                    if v_bias:
                        bv_sl = vec.tile([64, 1], f32, tag="bv", name="bv_sl")
                        nc.sync.dma_start(bv_sl[:], bv_d[l, h * D:(h + 1) * D][:, None])
                    if hh == 0:
                        dst = attnT[0:64, c * S:(c + 1) * S]
                        nc.vector.tensor_tensor(out=dst, in0=av[:, :],
                                                in1=bct[0:64, :], op=OP.mult)
                        if v_bias:
                            nc.vector.tensor_scalar(
                                out=dst,
                                in0=attnT[0:64, c * S:(c + 1) * S].bitcast(f32),
                                scalar1=bv_sl[:], scalar2=None, op0=OP.add)
                    else:
                        dst = attnT[64:128, c * S:(c + 1) * S]
                        nc.vector.tensor_tensor(out=dst, in0=av[:, :],
                                                in1=bct[0:64, :], op=OP.mult)
                        if v_bias:
                            nc.vector.tensor_scalar(
                                out=dst,
                                in0=attnT[64:128, c * S:(c + 1) * S].bitcast(f32),
                                scalar1=bv_sl[:], scalar2=None, op0=OP.add)

            # Wo projection (+bo) + residual -> y, LN1 per token tile
            y = acts.tile([128, TT * H], f32, tag="act", name=f"y_{l}")
            g1_bc, b1_bc = _ln_bcast(nc, pools, g1_d[l], b1_d[l], ln1_aff)
            if o_bias:
                bo_row = brow_p.tile([1, H], f32r, tag="br", name="bo_row")
                nc.sync.dma_start(bo_row[:], bo_d[l][None, :])
            woblks = []
            for n in range(2):
                wob = wbig.tile([128, HC, 384], f32r, tag="wb", name=f"wo_blk{n}")
                nc.sync.dma_start(wob[:], wo_d[l, n])
                woblks.append(wob)
            for tt in range(TT):
                for n in range(2):
                    po = psA.tile([128, 384], f32, tag="main", name="po")
                    if o_bias:
                        nc.tensor.matmul(po[:], lhsT=ones_sb[0:1, 0:128],
                                         rhs=bo_row[0:1, n * 384:(n + 1) * 384],
                                         start=True, stop=False)
                    for jc in range(HC):
                        nc.tensor.matmul(
                            po[:],
                            lhsT=attnT[:, jc * S + tt * 128: jc * S + tt * 128 + 128],
                            rhs=woblks[n][:, jc, :],
                            start=(not o_bias and jc == 0), stop=(jc == HC - 1))
                    sl = slice(tt * H + n * 384, tt * H + n * 384 + 384)
                    nc.vector.tensor_tensor(out=y[:, sl], in0=po[:, :],
                                            in1=x[:, sl], op=OP.add)
            _ln_phase(nc, pools, y, list(range(TT)), g1_bc, b1_bc)

            # yT
            yT = acts.tile([128, HC * S], f32r, tag="act", name=f"yT_{l}")
            _transpose_into(nc, pools, y, yT, ident)

            # FFN up: h1T = gelu(yT @ Wi + bi), hidden-major, bf16
            h1T = h1p.tile([128, FC * S], bf16, tag="h1", name=f"h1T_{l}")
            for fc in range(FC):
                wiblk = wsmall.tile([128, HC, 128], f32r, tag="ws", name="wi_blk")
                nc.sync.dma_start(wiblk[:], wi_d[l, fc])
                ph = psA.tile([128, S], f32, tag="main", name="ph")
                for ic in range(HC):
                    nc.tensor.matmul(ph[:], lhsT=wiblk[:, ic, :],
                                     rhs=yT[:, ic * S:(ic + 1) * S],
                                     start=(ic == 0), stop=(ic == HC - 1))
                if i_bias:
                    bi_sl = vec.tile([128, 1], f32, tag="v", name="bi_sl")
                    nc.sync.dma_start(bi_sl[:], bi_d[l, fc * 128:(fc + 1) * 128][:, None])
                    nc.scalar.activation(h1T[:, fc * S:(fc + 1) * S], ph[:], AF.Gelu,
                                         bias=bi_sl[:])
                else:
                    nc.scalar.activation(h1T[:, fc * S:(fc + 1) * S], ph[:], AF.Gelu)

            # FFN down (bf16) + bd + residual -> ffnout; waves of 4 (tt,n) pairs
            ffnout = acts.tile([128, TT * H], f32, tag="act", name=f"ffnout_{l}")
            g2_bc, b2_bc = _ln_bcast(nc, pools, g2_d[l], b2_d[l], ln2_aff)
            if d_bias:
                bd_row = brow_p.tile([1, H], f32r, tag="br", name="bd_row")
                nc.sync.dma_start(bd_row[:], bd_d[l][None, :])
            for wave in range(2):
                tts = (0, 1) if wave == 0 else (2, 3)
                wave_pairs = [(tt, n) for tt in tts for n in range(2)]
                accs = {}
                for (tt, n) in wave_pairs:
                    acc = psA.tile([128, 384], f32, tag="main", name=f"acc{tt}_{n}")
                    if d_bias:
                        nc.tensor.matmul(acc[:], lhsT=ones_sb[0:1, 0:128],
                                         rhs=bd_row[0:1, n * 384:(n + 1) * 384],
                                         start=True, stop=False)
                    accs[(tt, n)] = acc
                for fp in range(FC // 2):
                    wdblk = wdp.tile([128, 2, H], bf16, tag="wd", name="wd_blk")
                    nc.sync.dma_start(wdblk[:], wd_d[l, fp])
                    for two in range(2):
                        fc = 2 * fp + two
                        for (tt, n) in wave_pairs:
                            nc.tensor.matmul(
                                accs[(tt, n)][:],
                                lhsT=h1T[:, fc * S + tt * 128: fc * S + tt * 128 + 128],
                                rhs=wdblk[:, two, n * 384:(n + 1) * 384],
                                start=(not d_bias and fc == 0), stop=(fc == FC - 1))
                for tt in tts:
                    for n in range(2):
                        sl = slice(tt * H + n * 384, tt * H + n * 384 + 384)
                        nc.vector.tensor_tensor(out=ffnout[:, sl],
                                                in0=accs[(tt, n)][:, :],
                                                in1=y[:, sl], op=OP.add)
                _ln_phase(nc, pools, ffnout, list(tts), g2_bc, b2_bc)
            x = ffnout

        for tt in range(TT):
            nc.sync.dma_start(out_d[tt * 128:(tt + 1) * 128, :],
                              x[:, tt * H:(tt + 1) * H])

    nc.compile()
    return nc


def _prep_inputs(inputs, b):
    f = np.float32
    Wq, Wk, Wv, Wo, Wi = (np.asarray(inputs[k], f) for k in ("Wq", "Wk", "Wv", "Wo", "Wi"))
    WqS = np.ascontiguousarray(Wq.reshape(L, HC, 128, HC, 128).transpose(0, 3, 2, 1, 4))
    WkS = np.ascontiguousarray(Wk.reshape(L, HC, 128, HC, 128).transpose(0, 3, 2, 1, 4))
    WvS = np.ascontiguousarray(Wv.reshape(L, HC, 128, 2, 384).transpose(0, 3, 2, 1, 4))
    WoS = np.ascontiguousarray(Wo.reshape(L, HC, 128, 2, 384).transpose(0, 3, 2, 1, 4))
    WiS = np.ascontiguousarray(Wi.reshape(L, HC, 128, FC, 128).transpose(0, 3, 2, 1, 4))
    Wd = np.asarray(inputs["Wd"], f)
    # [L, I, H] -> [L, FC//2, 128, 2, H]: pairs of 128-row blocks, bf16
    WdB = np.ascontiguousarray(
        Wd.reshape(L, FC // 2, 2, 128, H).transpose(0, 1, 3, 2, 4)
    ).astype(ml_dtypes.bfloat16)
    mask = np.asarray(inputs["input_mask"], f)
    tti = np.asarray(inputs["token_type_ids"], np.int32)
    flags = dict(
        qk_bias=bool(np.any(np.asarray(inputs["bq"])) or np.any(np.asarray(inputs["bk"]))),
        v_bias=bool(np.any(np.asarray(inputs["bv"]))),
        o_bias=bool(np.any(np.asarray(inputs["bo"]))),
        i_bias=bool(np.any(np.asarray(inputs["bi"]))),
        d_bias=bool(np.any(np.asarray(inputs["bd"]))),
        ln1_aff=bool(np.any(np.asarray(inputs["ln1_g"]) != 1.0) or
                     np.any(np.asarray(inputs["ln1_b"]))),
        ln2_aff=bool(np.any(np.asarray(inputs["ln2_g"]) != 1.0) or
                     np.any(np.asarray(inputs["ln2_b"]))),
        emb_aff=bool(np.any(np.asarray(inputs["emb_ln_g"]) != 1.0) or
                     np.any(np.asarray(inputs["emb_ln_b"]))),
        use_mask=bool(np.any(mask != 1.0)),
        use_type=bool(np.any(tti != 0)),
    )
    pos_eff = np.asarray(inputs["pos_emb"], f)[:S]
    if not flags["use_type"]:
        # uniform type ids: fold type_emb[row0] into the position embedding
        pos_eff = pos_eff + np.asarray(inputs["type_emb"], f)[int(tti.flat[0])][None, :]
    shared = dict(
        tok_emb=np.asarray(inputs["tok_emb"], f),
        pos_emb=pos_eff,
        type_emb=np.asarray(inputs["type_emb"], f),
        emb_g=np.asarray(inputs["emb_ln_g"], f),
        emb_b=np.asarray(inputs["emb_ln_b"], f),
        WqS=WqS, WkS=WkS, WvS=WvS, WoS=WoS, WiS=WiS, WdB=WdB,
        bq=np.asarray(inputs["bq"], f), bk=np.asarray(inputs["bk"], f),
        bv=np.asarray(inputs["bv"], f), bo=np.asarray(inputs["bo"], f),
        bi=np.asarray(inputs["bi"], f), bd=np.asarray(inputs["bd"], f),
        ln1_g=np.asarray(inputs["ln1_g"], f), ln1_b=np.asarray(inputs["ln1_b"], f),
        ln2_g=np.asarray(inputs["ln2_g"], f), ln2_b=np.asarray(inputs["ln2_b"], f),
        ones=np.ones((128, 128), f),
        ident=np.eye(128, dtype=f),
    )
    in_maps = []
    ids = np.asarray(inputs["input_ids"], np.int32)
    for c in range(b):
        m = dict(shared)
        m["ids"] = np.ascontiguousarray(ids[c])
        m["tti"] = np.ascontiguousarray(tti[c])
        m["mb"] = np.ascontiguousarray((1.0 - mask[c]) * -10000.0)
        in_maps.append(m)
    return in_maps, flags


def kernel(**inputs):
    global LAST_EXEC_TIME_NS
    n_layers = int(os.environ.get("BERT_LAYERS", L))
    trace = bool(os.environ.get("BERT_TRACE"))
    in_maps, flags = _prep_inputs(inputs, B)
    nc = build(n_layers, flags)
    res = bass_utils.run_bass_kernel_spmd(
        nc, in_maps, core_ids=list(range(B)), trace=trace)
    LAST_EXEC_TIME_NS = res.exec_time_ns
    out = np.stack([res.results[c]["out"] for c in range(B)])
    return out.astype(np.float32)


# revision 8
# speedup vs baseline: 1.8491x; 1.1815x over previous
"""BERT-base forward on 8 Trainium2 NeuronCores, data-parallel over batch.

Each core runs the full 12-layer model on one batch element (512 tokens).
Activations live in SBUF for the whole forward pass; weights stream from HBM.
Big matmuls run in float32r (fp32 data on the fast PE path, N>=256); the
FFN-down matmul runs in bf16 to halve its weight streaming.

Layouts per core (SBUF tiles are [128 partitions, free]):
  token-major  x/y/ffnout: [128 tok, 4*768]   (col block tt = token tile)
  hidden-major xT/QT/KT/attnT/yT: [128 hid, 6*512] (col block c = hidden chunk)
  V token-major [128 tok, 4*768]; h1T hidden-major [128 f, 24*512] bf16.

Attention (per head pair c: heads 2c at partitions 0:64, 2c+1 at 64:128):
  S^T[k,q] = matmul(lhsT=KT[d,k-tile], rhs=QT[d,q]) row-packed pairs
  expS = Exp(S^T/8 + maskbias_k)  (no max subtraction: |scores/8| < 3)
  denom via matmul(lhsT=ones[128,128]) -> sums broadcast across partitions
  O^T = matmul(lhsT=V[:,head cols], rhs=expS) accumulated over k chunks,
  then normalized by 1/sums (+bv) at eviction.

Work that is provably a no-op for the given inputs (zero biases, unit
gammas, zero betas, all-ones mask) is skipped at build time; the general
path stays available and is selected per-input on the host.
"""
import os
import numpy as np
import ml_dtypes
from contextlib import ExitStack

import concourse.bass as bass
import concourse.tile as tile
from concourse import bacc, mybir
from concourse import bass_utils

f32 = mybir.dt.float32
f32r = mybir.dt.float32r
bf16 = mybir.dt.bfloat16
i32 = mybir.dt.int32
AF = mybir.ActivationFunctionType
OP = mybir.AluOpType
AX = mybir.AxisListType

V, H, L, NH, I, P, B, S = 30000, 768, 12, 12, 3072, 512, 8, 512
D = H // NH          # 64
HC = H // 128        # 6 hidden chunks
FC = I // 128        # 24 ffn chunks
TT = S // 128        # 4 token tiles
LN_EPS = 1e-3

LAST_EXEC_TIME_NS = None


def _ln_phase(nc, pools, z, tts, g_bc, b_bc):
    """LN over hidden dim on token tiles `tts` of z (in place).

    var = E[x^2] - mu^2 so the Square pass runs concurrently with the
    row-sum; ACT ops are batched per function to avoid table reloads.
    """
    vec, scratch = pools["vec"], pools["scratch"]
    sls = {tt: slice(tt * H, (tt + 1) * H) for tt in tts}
    s, ssq, sd, rstd, b2, mr = {}, {}, {}, {}, {}, {}
    for tt in tts:
        s[tt] = vec.tile([128, 1], f32, tag="v", name=f"ln_s{tt}")
        nc.vector.reduce_sum(out=s[tt][:], in_=z[:, sls[tt]], axis=AX.X)
    for tt in tts:
        sq = scratch.tile([128, H], f32, tag="sc", name="ln_sq")
        ssq[tt] = vec.tile([128, 1], f32, tag="v", name=f"ln_ssq{tt}")
        nc.scalar.activation(sq[:], z[:, sls[tt]], AF.Square, accum_out=ssq[tt][:])
    for tt in tts:
        b2[tt] = vec.tile([128, 1], f32, tag="v", name=f"ln_b2{tt}")
        nc.vector.tensor_scalar(out=b2[tt][:], in0=s[tt][:], scalar1=s[tt][:],
                                scalar2=float(-1.0 / (H * H)), op0=OP.mult,
                                op1=OP.mult)
        nc.vector.tensor_scalar(out=b2[tt][:], in0=b2[tt][:], scalar1=float(LN_EPS),
                                scalar2=None, op0=OP.add)
    for tt in tts:
        sd[tt] = vec.tile([128, 1], f32, tag="v", name=f"ln_sd{tt}")
        nc.scalar.activation(sd[tt][:], ssq[tt][:], AF.Sqrt, bias=b2[tt][:],
                             scale=1.0 / H)
    for tt in tts:
        rstd[tt] = vec.tile([128, 1], f32, tag="v", name=f"ln_rstd{tt}")
        nc.vector.reciprocal(rstd[tt][:], sd[tt][:])
        mr[tt] = vec.tile([128, 1], f32, tag="v", name=f"ln_mr{tt}")
        nc.vector.tensor_scalar(out=mr[tt][:], in0=s[tt][:], scalar1=rstd[tt][:],
                                scalar2=float(-1.0 / H), op0=OP.mult, op1=OP.mult)
    for tt in tts:
        nc.vector.tensor_scalar(out=z[:, sls[tt]], in0=z[:, sls[tt]],
                                scalar1=rstd[tt][:], scalar2=mr[tt][:],
                                op0=OP.mult, op1=OP.add)
        if g_bc is not None:
            nc.vector.tensor_tensor(out=z[:, sls[tt]], in0=z[:, sls[tt]],
                                    in1=g_bc[:], op=OP.mult)
        if b_bc is not None:
            nc.vector.tensor_tensor(out=z[:, sls[tt]], in0=z[:, sls[tt]],
                                    in1=b_bc[:], op=OP.add)


def _ln_bcast(nc, pools, g_row, b_row, affine):
    if not affine:
        return None, None
    gb = pools["gb"]
    g_bc = gb.tile([128, H], f32, tag="gb", name="g_bc")
    nc.sync.dma_start(g_bc[:], g_row[None, :].partition_broadcast(128))
    b_bc = gb.tile([128, H], f32, tag="gb", name="b_bc")
    nc.sync.dma_start(b_bc[:], b_row[None, :].partition_broadcast(128))
    return g_bc, b_bc


def _transpose_into(nc, pools, src, dst, ident):
    """src token-major [128, 4*768] -> dst hidden-major [128, 6*512] f32r."""
    psT = pools["psT"]
    for c in range(HC):
        tp = psT.tile([128, S], f32, tag="tp", name="tp")
        for tt in range(TT):
            nc.tensor.transpose(tp[:, tt * 128:(tt + 1) * 128],
                                src[:, tt * H + c * 128: tt * H + c * 128 + 128],
                                ident[:])
        nc.vector.tensor_copy(dst[:, c * S:(c + 1) * S], tp[:])


def build(n_layers=L, flags=None):
    fl = flags or {}
    qk_bias = fl.get("qk_bias", True)
    v_bias = fl.get("v_bias", True)
    o_bias = fl.get("o_bias", True)
    i_bias = fl.get("i_bias", True)
    d_bias = fl.get("d_bias", True)
    ln1_aff = fl.get("ln1_aff", True)
    ln2_aff = fl.get("ln2_aff", True)
    emb_aff = fl.get("emb_aff", True)
    use_mask = fl.get("use_mask", True)
    use_type = fl.get("use_type", True)

    nc = bacc.Bacc("TRN2", target_bir_lowering=False, debug=False, num_devices=8)

    dt_in = lambda n, s, d: nc.dram_tensor(n, s, d, kind="ExternalInput").ap()
    ids_d = dt_in("ids", [S], i32)
    tti_d = dt_in("tti", [S], i32)
    mb_d = dt_in("mb", [S], f32)
    tok_d = dt_in("tok_emb", [V, H], f32)
    pos_d = dt_in("pos_emb", [S, H], f32)
    typ_d = dt_in("type_emb", [2, H], f32)
    eg_d = dt_in("emb_g", [H], f32)
    eb_d = dt_in("emb_b", [H], f32)
    wq_d = dt_in("WqS", [L, HC, 128, HC, 128], f32r)
    wk_d = dt_in("WkS", [L, HC, 128, HC, 128], f32r)
    wv_d = dt_in("WvS", [L, 2, 128, HC, 384], f32r)
    wo_d = dt_in("WoS", [L, 2, 128, HC, 384], f32r)
    wi_d = dt_in("WiS", [L, FC, 128, HC, 128], f32r)
    wd_d = dt_in("WdB", [L, FC // 2, 128, 2, H], bf16)
    bq_d = dt_in("bq", [L, H], f32)
    bk_d = dt_in("bk", [L, H], f32)
    bv_d = dt_in("bv", [L, H], f32)
    bo_d = dt_in("bo", [L, H], f32r)
    bi_d = dt_in("bi", [L, I], f32)
    bd_d = dt_in("bd", [L, H], f32r)
    g1_d = dt_in("ln1_g", [L, H], f32)
    b1_d = dt_in("ln1_b", [L, H], f32)
    g2_d = dt_in("ln2_g", [L, H], f32)
    b2_d = dt_in("ln2_b", [L, H], f32)
    ones_d = dt_in("ones", [128, 128], f32r)
    ident_d = dt_in("ident", [128, 128], f32)
    out_d = nc.dram_tensor("out", [S, H], f32, kind="ExternalOutput").ap()

    with tile.TileContext(nc) as tc, ExitStack() as ctx:
        acts = ctx.enter_context(tc.tile_pool(name="acts", bufs=7))
        h1p = ctx.enter_context(tc.tile_pool(name="h1p", bufs=1))
        wbig = ctx.enter_context(tc.tile_pool(name="wbig", bufs=2))
        wsmall = ctx.enter_context(tc.tile_pool(name="wsmall", bufs=3))
        wdp = ctx.enter_context(tc.tile_pool(name="wdp", bufs=4))
        gb = ctx.enter_context(tc.tile_pool(name="gb", bufs=2))
        exps_p = ctx.enter_context(tc.tile_pool(name="exps_p", bufs=12))
        bc_p = ctx.enter_context(tc.tile_pool(name="bc_p", bufs=2))
        avtmp_p = ctx.enter_context(tc.tile_pool(name="avtmp_p", bufs=2))
        scratch = ctx.enter_context(tc.tile_pool(name="scratch", bufs=2))
        vec = ctx.enter_context(tc.tile_pool(name="vec", bufs=28))
        brow_p = ctx.enter_context(tc.tile_pool(name="brow_p", bufs=1))
        const = ctx.enter_context(tc.tile_pool(name="const", bufs=1))
        psA = ctx.enter_context(tc.tile_pool(name="psA", bufs=6, space="PSUM"))
        psT = ctx.enter_context(tc.tile_pool(name="psT", bufs=2, space="PSUM"))
        pools = dict(gb=gb, vec=vec, scratch=scratch, psT=psT)

        # constants
        ones_sb = const.tile([128, 128], f32r, tag="ones", name="ones_sb")
        nc.sync.dma_start(ones_sb[:], ones_d[:])
        ident = const.tile([128, 128], f32, tag="ident", name="ident")
        nc.sync.dma_start(ident[:], ident_d[:])
        eps_t = const.tile([128, 1], f32, tag="eps", name="eps_t")
        nc.vector.memset(eps_t[:], LN_EPS)
        pools["eps"] = eps_t
        ids_sb = const.tile([128, TT], i32, tag="ids", name="ids_sb")
        nc.sync.dma_start(ids_sb[:], ids_d.rearrange("(t p) -> p t", p=128))
        if use_type:
            tti_sb = const.tile([128, TT], i32, tag="tti", name="tti_sb")
            nc.sync.dma_start(tti_sb[:], tti_d.rearrange("(t p) -> p t", p=128))
        if use_mask:
            mb_sb = const.tile([128, TT], f32, tag="mb", name="mb_sb")
            nc.sync.dma_start(mb_sb[:], mb_d.rearrange("(t p) -> p t", p=128))

        # ---- embedding ----
        x = acts.tile([128, TT * H], f32, tag="act", name="x_emb")
        eg_bc, eb_bc = _ln_bcast(nc, pools, eg_d, eb_d, emb_aff)
        for tt in range(TT):
            sl = slice(tt * H, (tt + 1) * H)
            nc.gpsimd.indirect_dma_start(
                out=x[:, sl], out_offset=None, in_=tok_d[:],
                in_offset=bass.IndirectOffsetOnAxis(ap=ids_sb[:, tt:tt + 1], axis=0))
            if use_type:
                tmp_t = gb.tile([128, H], f32, tag="gb", name="emb_tmp")
                nc.gpsimd.indirect_dma_start(
                    out=tmp_t[:], out_offset=None, in_=typ_d[:],
                    in_offset=bass.IndirectOffsetOnAxis(ap=tti_sb[:, tt:tt + 1], axis=0))
                nc.vector.tensor_tensor(out=x[:, sl], in0=x[:, sl], in1=tmp_t[:],
                                        op=OP.add)
            tmp_p = gb.tile([128, H], f32, tag="gb", name="emb_pos")
            nc.sync.dma_start(tmp_p[:], pos_d[tt * 128:(tt + 1) * 128, :])
            nc.vector.tensor_tensor(out=x[:, sl], in0=x[:, sl], in1=tmp_p[:], op=OP.add)
        _ln_phase(nc, pools, x, list(range(TT)), eg_bc, eb_bc)

        # ---- layers ----
        for l in range(n_layers):
            xT = acts.tile([128, HC * S], f32r, tag="act", name=f"xT_{l}")
            _transpose_into(nc, pools, x, xT, ident)

            # Q^T, K^T hidden-major
            QT = acts.tile([128, HC * S], f32r, tag="act", name=f"QT_{l}")
            KT = acts.tile([128, HC * S], f32r, tag="act", name=f"KT_{l}")
            for dst, w_d, b_d in ((QT, wq_d, bq_d), (KT, wk_d, bk_d)):
                for j in range(HC):
                    wblk = wsmall.tile([128, HC, 128], f32r, tag="ws", name="wqk_blk")
                    nc.sync.dma_start(wblk[:], w_d[l, j])
                    pq = psA.tile([128, S], f32, tag="main", name="pq")
                    for ic in range(HC):
                        nc.tensor.matmul(pq[:], lhsT=wblk[:, ic, :],
                                         rhs=xT[:, ic * S:(ic + 1) * S],
                                         start=(ic == 0), stop=(ic == HC - 1))
                    jsl = slice(j * S, (j + 1) * S)
                    if qk_bias:
                        b_sl = vec.tile([128, 1], f32, tag="v", name="bqk_sl")
                        nc.sync.dma_start(b_sl[:], b_d[l, j * 128:(j + 1) * 128][:, None])
                        nc.scalar.activation(dst[:, jsl], pq[:], AF.Identity,
                                             bias=b_sl[:])
                    else:
                        nc.vector.tensor_copy(dst[:, jsl], pq[:])

            # V token-major
            Vt = acts.tile([128, TT * H], f32r, tag="act", name=f"V_{l}")
            for n in range(2):
                wvblk = wbig.tile([128, HC, 384], f32r, tag="wb", name="wv_blk")
                nc.sync.dma_start(wvblk[:], wv_d[l, n])
                for tt in range(TT):
                    pv = psA.tile([128, 384], f32, tag="main", name="pv")
                    for ic in range(HC):
                        nc.tensor.matmul(
                            pv[:], lhsT=xT[:, ic * S + tt * 128: ic * S + tt * 128 + 128],
                            rhs=wvblk[:, ic, :],
                            start=(ic == 0), stop=(ic == HC - 1))
                    nc.vector.tensor_copy(
                        Vt[:, tt * H + n * 384: tt * H + n * 384 + 384], pv[:])

            # attention, head pairs
            attnT = acts.tile([128, HC * S], f32r, tag="act", name=f"attnT_{l}")
            for c in range(HC):
                es = [[None] * TT for _ in range(2)]
                for kc in range(TT):
                    for hh in range(2):
                        r0 = 64 * hh
                        sp = psA.tile([128, S], f32, tag="main", name="sp")
                        nc.tensor.matmul(
                            sp[:],
                            lhsT=KT[r0:r0 + 64, c * S + kc * 128: c * S + kc * 128 + 128],
                            rhs=QT[r0:r0 + 64, c * S:(c + 1) * S],
                            start=True, stop=True)
                        e = exps_p.tile([128, S], f32r, tag="e", name=f"e{hh}_{kc}")
                        mbias = mb_sb[:, kc:kc + 1] if use_mask else 0.0
                        nc.scalar.activation(e[:], sp[:], AF.Exp,
                                             bias=mbias, scale=0.125)
                        es[hh][kc] = e
                for hh in range(2):
                    h = 2 * c + hh
                    ssum = psA.tile([128, S], f32, tag="main", name="ssum")
                    for kc in range(TT):
                        nc.tensor.matmul(ssum[:], lhsT=ones_sb[:, 0:128],
                                         rhs=es[hh][kc][:],
                                         start=(kc == 0), stop=(kc == TT - 1))
                    bct = bc_p.tile([128, S], f32, tag="bc", name="bct")
                    nc.vector.reciprocal_approx_fast(out=bct[0:64, :],
                                                     in_=ssum[0:64, :])
                    av = psA.tile([64, S], f32, tag="main", name="av")
                    for kc in range(TT):
                        nc.tensor.matmul(
                            av[:], lhsT=Vt[:, kc * H + h * D: kc * H + h * D + D],
                            rhs=es[hh][kc][:],
                            start=(kc == 0), stop=(kc == TT - 1))
                    if v_bias:
                        bv_sl = vec.tile([64, 1], f32, tag="bv", name="bv_sl")
                        nc.sync.dma_start(bv_sl[:], bv_d[l, h * D:(h + 1) * D][:, None])
                    if hh == 0:
                        dst = attnT[0:64, c * S:(c + 1) * S]
                        nc.vector.tensor_tensor(out=dst, in0=av[:, :],
                                                in1=bct[0:64, :], op=OP.mult)
                        if v_bias:
                            nc.vector.tensor_scalar(
                                out=dst,
                                in0=attnT[0:64, c * S:(c + 1) * S].bitcast(f32),
                                scalar1=bv_sl[:], scalar2=None, op0=OP.add)
                    else:
                        dst = attnT[64:128, c * S:(c + 1) * S]
                        nc.vector.tensor_tensor(out=dst, in0=av[:, :],
                                                in1=bct[0:64, :], op=OP.mult)
                        if v_bias:
                            nc.vector.tensor_scalar(
                                out=dst,
                                in0=attnT[64:128, c * S:(c + 1) * S].bitcast(f32),
                                scalar1=bv_sl[:], scalar2=None, op0=OP.add)

            # Wo projection (+bo) + residual -> y, LN1 per token tile
            y = acts.tile([128, TT * H], f32, tag="act", name=f"y_{l}")
            g1_bc, b1_bc = _ln_bcast(nc, pools, g1_d[l], b1_d[l], ln1_aff)
            if o_bias:
                bo_row = brow_p.tile([1, H], f32r, tag="br", name="bo_row")
                nc.sync.dma_start(bo_row[:], bo_d[l][None, :])
            woblks = []
            for n in range(2):
                wob = wbig.tile([128, HC, 384], f32r, tag="wb", name=f"wo_blk{n}")
                nc.sync.dma_start(wob[:], wo_d[l, n])
                woblks.append(wob)
            for tt in range(TT):
                for n in range(2):
                    po = psA.tile([128, 384], f32, tag="main", name="po")
                    if o_bias:
                        nc.tensor.matmul(po[:], lhsT=ones_sb[0:1, 0:128],
                                         rhs=bo_row[0:1, n * 384:(n + 1) * 384],
                                         start=True, stop=False)
                    for jc in range(HC):
                        nc.tensor.matmul(
                            po[:],
                            lhsT=attnT[:, jc * S + tt * 128: jc * S + tt * 128 + 128],
                            rhs=woblks[n][:, jc, :],
                            start=(not o_bias and jc == 0), stop=(jc == HC - 1))
                    sl = slice(tt * H + n * 384, tt * H + n * 384 + 384)
                    nc.vector.tensor_tensor(out=y[:, sl], in0=po[:, :],
                                            in1=x[:, sl], op=OP.add)
            _ln_phase(nc, pools, y, list(range(TT)), g1_bc, b1_bc)

            # yT
            yT = acts.tile([128, HC * S], f32r, tag="act", name=f"yT_{l}")
            _transpose_into(nc, pools, y, yT, ident)

            # FFN up: h1T = gelu(yT @ Wi + bi), hidden-major, bf16
            h1T = h1p.tile([128, FC * S], bf16, tag="h1", name=f"h1T_{l}")
            for fc in range(FC):
                wiblk = wsmall.tile([128, HC, 128], f32r, tag="ws", name="wi_blk")
                nc.sync.dma_start(wiblk[:], wi_d[l, fc])
                ph = psA.tile([128, S], f32, tag="main", name="ph")
                for ic in range(HC):
                    nc.tensor.matmul(ph[:], lhsT=wiblk[:, ic, :],
                                     rhs=yT[:, ic * S:(ic + 1) * S],
                                     start=(ic == 0), stop=(ic == HC - 1))
                if i_bias:
                    bi_sl = vec.tile([128, 1], f32, tag="v", name="bi_sl")
                    nc.sync.dma_start(bi_sl[:], bi_d[l, fc * 128:(fc + 1) * 128][:, None])
                    nc.scalar.activation(h1T[:, fc * S:(fc + 1) * S], ph[:], AF.Gelu,
                                         bias=bi_sl[:])
                else:
                    nc.scalar.activation(h1T[:, fc * S:(fc + 1) * S], ph[:], AF.Gelu)

            # FFN down (bf16) + bd + residual -> ffnout; waves of 4 (tt,n) pairs
            ffnout = acts.tile([128, TT * H], f32, tag="act", name=f"ffnout_{l}")
            g2_bc, b2_bc = _ln_bcast(nc, pools, g2_d[l], b2_d[l], ln2_aff)
            if d_bias:
                bd_row = brow_p.tile([1, H], f32r, tag="br", name="bd_row")
                nc.sync.dma_start(bd_row[:], bd_d[l][None, :])
            for wave in range(2):
                tts = (0, 1) if wave == 0 else (2, 3)
                wave_pairs = [(tt, n) for tt in tts for n in range(2)]
                accs = {}
                for (tt, n) in wave_pairs:
                    acc = psA.tile([128, 384], f32, tag="main", name=f"acc{tt}_{n}")
                    if d_bias:
                        nc.tensor.matmul(acc[:], lhsT=ones_sb[0:1, 0:128],
                                         rhs=bd_row[0:1, n * 384:(n + 1) * 384],
                                         start=True, stop=False)
                    accs[(tt, n)] = acc
                for fp in range(FC // 2):
                    wdblk = wdp.tile([128, 2, H], bf16, tag="wd", name="wd_blk")
                    nc.sync.dma_start(wdblk[:], wd_d[l, fp])
                    for two in range(2):
                        fc = 2 * fp + two
                        for (tt, n) in wave_pairs:
                            nc.tensor.matmul(
                                accs[(tt, n)][:],
                                lhsT=h1T[:, fc * S + tt * 128: fc * S + tt * 128 + 128],
                                rhs=wdblk[:, two, n * 384:(n + 1) * 384],
                                start=(not d_bias and fc == 0), stop=(fc == FC - 1))
                for tt in tts:
                    for n in range(2):
                        sl = slice(tt * H + n * 384, tt * H + n * 384 + 384)
                        nc.vector.tensor_tensor(out=ffnout[:, sl],
                                                in0=accs[(tt, n)][:, :],
                                                in1=y[:, sl], op=OP.add)
                _ln_phase(nc, pools, ffnout, list(tts), g2_bc, b2_bc)
            x = ffnout

        for tt in range(TT):
            nc.sync.dma_start(out_d[tt * 128:(tt + 1) * 128, :],
                              x[:, tt * H:(tt + 1) * H])

    nc.compile()
    return nc


def _prep_inputs(inputs, b):
    f = np.float32
    Wq, Wk, Wv, Wo, Wi = (np.asarray(inputs[k], f) for k in ("Wq", "Wk", "Wv", "Wo", "Wi"))
    WqS = np.ascontiguousarray(Wq.reshape(L, HC, 128, HC, 128).transpose(0, 3, 2, 1, 4))
    WkS = np.ascontiguousarray(Wk.reshape(L, HC, 128, HC, 128).transpose(0, 3, 2, 1, 4))
    WvS = np.ascontiguousarray(Wv.reshape(L, HC, 128, 2, 384).transpose(0, 3, 2, 1, 4))
    WoS = np.ascontiguousarray(Wo.reshape(L, HC, 128, 2, 384).transpose(0, 3, 2, 1, 4))
    WiS = np.ascontiguousarray(Wi.reshape(L, HC, 128, FC, 128).transpose(0, 3, 2, 1, 4))
    Wd = np.asarray(inputs["Wd"], f)
    # [L, I, H] -> [L, FC//2, 128, 2, H]: pairs of 128-row blocks, bf16
    WdB = np.ascontiguousarray(
        Wd.reshape(L, FC // 2, 2, 128, H).transpose(0, 1, 3, 2, 4)
    ).astype(ml_dtypes.bfloat16)
    mask = np.asarray(inputs["input_mask"], f)
    tti = np.asarray(inputs["token_type_ids"], np.int32)
    flags = dict(
        qk_bias=bool(np.any(np.asarray(inputs["bq"])) or np.any(np.asarray(inputs["bk"]))),
        v_bias=bool(np.any(np.asarray(inputs["bv"]))),
        o_bias=bool(np.any(np.asarray(inputs["bo"]))),
        i_bias=bool(np.any(np.asarray(inputs["bi"]))),
        d_bias=bool(np.any(np.asarray(inputs["bd"]))),
        ln1_aff=bool(np.any(np.asarray(inputs["ln1_g"]) != 1.0) or
                     np.any(np.asarray(inputs["ln1_b"]))),
        ln2_aff=bool(np.any(np.asarray(inputs["ln2_g"]) != 1.0) or
                     np.any(np.asarray(inputs["ln2_b"]))),
        emb_aff=bool(np.any(np.asarray(inputs["emb_ln_g"]) != 1.0) or
                     np.any(np.asarray(inputs["emb_ln_b"]))),
        use_mask=bool(np.any(mask != 1.0)),
        use_type=bool(np.any(tti != 0)),
    )
    pos_eff = np.asarray(inputs["pos_emb"], f)[:S]
    if not flags["use_type"]:
        # uniform type ids: fold type_emb[row0] into the position embedding
        pos_eff = pos_eff + np.asarray(inputs["type_emb"], f)[int(tti.flat[0])][None, :]
    shared = dict(
        tok_emb=np.asarray(inputs["tok_emb"], f),
        pos_emb=pos_eff,
        type_emb=np.asarray(inputs["type_emb"], f),
        emb_g=np.asarray(inputs["emb_ln_g"], f),
        emb_b=np.asarray(inputs["emb_ln_b"], f),
        WqS=WqS, WkS=WkS, WvS=WvS, WoS=WoS, WiS=WiS, WdB=WdB,
        bq=np.asarray(inputs["bq"], f), bk=np.asarray(inputs["bk"], f),
        bv=np.asarray(inputs["bv"], f), bo=np.asarray(inputs["bo"], f),
        bi=np.asarray(inputs["bi"], f), bd=np.asarray(inputs["bd"], f),
        ln1_g=np.asarray(inputs["ln1_g"], f), ln1_b=np.asarray(inputs["ln1_b"], f),
        ln2_g=np.asarray(inputs["ln2_g"], f), ln2_b=np.asarray(inputs["ln2_b"], f),
        ones=np.ones((128, 128), f),
        ident=np.eye(128, dtype=f),
    )
    in_maps = []
    ids = np.asarray(inputs["input_ids"], np.int32)
    for c in range(b):
        m = dict(shared)
        m["ids"] = np.ascontiguousarray(ids[c])
        m["tti"] = np.ascontiguousarray(tti[c])
        m["mb"] = np.ascontiguousarray((1.0 - mask[c]) * -10000.0)
        in_maps.append(m)
    return in_maps, flags


def kernel(**inputs):
    global LAST_EXEC_TIME_NS
    n_layers = int(os.environ.get("BERT_LAYERS", L))
    trace = bool(os.environ.get("BERT_TRACE"))
    in_maps, flags = _prep_inputs(inputs, B)
    nc = build(n_layers, flags)
    res = bass_utils.run_bass_kernel_spmd(
        nc, in_maps, core_ids=list(range(B)), trace=trace)
    LAST_EXEC_TIME_NS = res.exec_time_ns
    out = np.stack([res.results[c]["out"] for c in range(B)])
    return out.astype(np.float32)
